# revision 1
# baseline (speedup 1.0000x reference)
"""Trainium2 Bass kernel for BayesianChangePointDetector (segment_reduce).

Contract: kernel(**inputs) takes FULL inputs (x:[128,8192,32] f32, plus 3
scalar prior params) and returns the FULL [128] f32 output. Internally the
batch dim is sharded across 8 NeuronCores (16 rows each, pure data parallel,
no collectives), each core runs the same Bass/Tile program, and the host
concatenates the 8 per-core [16] outputs.

Per-core layout: partition p in [0,128) owns t in [64p, 64p+64); the free dim
is (b, u) with b in [0,16) batch rows and u in [0,64). The heavy pass is a
single DVE reduce over N=32; prefix sums use the native tensor_tensor_scan
plus a cross-partition carry fixed up with a triangular-ones matmul on PE.
"""

import sys

if "/opt/trn_rl_repo" not in sys.path:
    sys.path.insert(0, "/opt/trn_rl_repo")

import math
from contextlib import ExitStack

import numpy as np

import concourse.bass as bass
import concourse.tile as tile
from concourse import mybir

F32 = mybir.dt.float32
AF = mybir.ActivationFunctionType
ALU = mybir.AluOpType
AX = mybir.AxisListType

B, T, N = 128, 8192, 32
NCORES = 8
BL = B // NCORES  # 16 batch rows per core
P = 128           # partitions = t-blocks
U = T // P        # 64 t's per partition
BC = 4            # batch rows per processing chunk
NCHUNK = BL // BC
UF = 32           # u-range whose n-fold (32->16) runs on gpsimd
UF_SCHED = [16, 32, 32, 32]  # per-chunk fold; small for chunk0 (early DVE start)
CHUNK_SIZES = [4, 4, 4, 4]   # batch rows per chunk (uniform won all model sweeps)
XP_BUFS = 2       # x staging double-buffer depth
WK_BUFS = 3       # per-chunk work tile depth
NS = 32           # scalar-slot count
NEG = -1.0e30

# near-end threshold: P_split > 6553  <=>  g >= 6553 (g = P_split-1 = 64p+u)
NE_P0 = 6553 // U          # 102
NE_U0 = 6553 - NE_P0 * U   # 25
# valid candidates: P_split in [16, 8176) <=> g in [15, 8175)
LO_INV_U = 15              # g<15 -> p==0, u<15 invalid
HI_INV_U = 8174 - 127 * U + 1  # g>8174 -> p==127, u>=47 invalid


def build_body(ctx, tc, x, params, utc, idc, out, pm_zero=False):
    nc = tc.nc
    pers = ctx.enter_context(tc.tile_pool(name="pers", bufs=1))
    xp = ctx.enter_context(tc.tile_pool(name="xp", bufs=XP_BUFS))
    wk = ctx.enter_context(tc.tile_pool(name="wk", bufs=WK_BUFS))
    psp = ctx.enter_context(tc.tile_pool(name="psp", bufs=2, space="PSUM"))
    ps1 = ctx.enter_context(tc.tile_pool(name="ps1", bufs=1, space="PSUM"))

    # ---------- small const DMAs first (same SP ring, FIFO ahead of x),
    # then x DMAs split in u-halves for earlier reduce start ----------
    ut_t = pers.tile([P, P], F32)     # strictly-upper triangular ones (q<m)
    ones_t = pers.tile([P, P], F32)   # all-ones
    id_t = pers.tile([P, P], F32)     # identity (PE transpose)
    gt = pers.tile([P, U], F32)       # g = 64p+u
    ptile = pers.tile([P, 3], F32)
    nc.sync.dma_start(ptile[:], params[:])
    nc.gpsimd.memset(ones_t[:], 1.0)
    gti = pers.tile([P, U], mybir.dt.int32)
    nc.gpsimd.iota(gti[:], [[1, U]], base=0, channel_multiplier=U)
    nc.vector.tensor_copy(gt[:], gti[:])

    chunks = []
    o = 0
    for c in CHUNK_SIZES:
        chunks.append((o, c))
        o += c
    assert o == BL
    xts = []
    for ci, (bs, bc) in enumerate(chunks):
        xt = xp.tile([P, bc, U, N], F32, tag="xt")
        src = x[bs : bs + bc].rearrange("b (p u) n -> p b u n", p=P)
        if ci == 0:
            QU = U // 4
            for q in range(4):
                nc.sync.dma_start(
                    xt[:, :, q * QU : (q + 1) * QU, :],
                    src[:, :, q * QU : (q + 1) * QU, :],
                )
        else:
            HU = U // 2
            nc.sync.dma_start(xt[:, :, 0:HU, :], src[:, :, 0:HU, :])
            nc.sync.dma_start(xt[:, :, HU:U, :], src[:, :, HU:U, :])
        if ci == 0:
            # needed from the first carry matmul / finale only; land behind chunk0
            nc.sync.dma_start(ut_t[:], utc[:])
            nc.sync.dma_start(id_t[:], idc[:])
        xts.append(xt)

    # scalar slots, computed redundantly on all 128 partitions
    sv = pers.tile([P, NS], F32)
    tmp = pers.tile([P, 8], F32)

    def s(i):
        return sv[:, i : i + 1]

    def tm(i):
        return tmp[:, i : i + 1]

    # ---------- scalar prep on partition 0 ----------
    # slots: 0 pm, 1 inv_nv, 2 inv_pv, 3 neg_inv_nv, 4 zRb, 5 k, 6 c,
    # 7 -kq/2, 8 k^2/2, 9 kq/2, 10 c*k, 11 c^2/2, 12 sc, 13 pvW,
    # 14 L2pinv, 15 Lpv, 16 LpvW, 17 8192*inv_nv, 18 inv_nv/8192,
    # 19 bfWc, 20 pv, 21 nv, 22 pm^2*inv_pv, 23 -4096*L2pinv
    # softplus(x) = ln(1 + exp(x)); Exp+Ln share one ACT table set
    nc.scalar.activation(tm(0), ptile[:, 1:2], AF.Exp)
    nc.vector.tensor_scalar_add(tm(0), tm(0), 1.0)
    nc.scalar.activation(s(20), tm(0), AF.Ln)
    nc.scalar.activation(tm(1), ptile[:, 2:3], AF.Exp)
    nc.vector.tensor_scalar_add(tm(1), tm(1), 1.0)
    nc.scalar.activation(s(21), tm(1), AF.Ln)
    nc.vector.tensor_copy(s(0), ptile[:, 0:1])
    nc.vector.reciprocal(s(1), s(21))
    nc.vector.reciprocal(s(2), s(20))
    nc.vector.tensor_scalar_mul(s(3), s(1), -1.0)
    nc.vector.tensor_scalar(s(4), s(1), 8191.0, s(2), ALU.mult, ALU.add)
    nc.vector.tensor_scalar_mul(s(5), s(1), 1.0 / 32.0)
    nc.vector.tensor_mul(s(6), s(0), s(2))
    nc.vector.tensor_scalar_mul(s(7), s(1), -0.5 / 1024.0)
    nc.vector.tensor_scalar_mul(s(9), s(1), 0.5 / 1024.0)
    nc.vector.tensor_mul(tm(0), s(5), s(5))
    nc.vector.tensor_scalar_mul(s(8), tm(0), 0.5)
    nc.vector.tensor_mul(s(10), s(6), s(5))
    nc.vector.tensor_mul(tm(1), s(6), s(6))
    nc.vector.tensor_scalar_mul(s(11), tm(1), 0.5)
    nc.scalar.activation(s(14), s(21), AF.Ln, scale=2.0 * math.pi)
    nc.scalar.activation(s(15), s(20), AF.Ln)
    nc.vector.tensor_scalar_mul(s(17), s(1), 8192.0)
    nc.vector.tensor_scalar(tm(2), s(1), 8192.0, s(2), ALU.mult, ALU.add)
    nc.vector.reciprocal(s(13), tm(2))
    nc.scalar.activation(s(16), s(13), AF.Ln)
    nc.vector.tensor_scalar_mul(s(18), s(1), 1.0 / 8192.0)
    nc.vector.tensor_mul(tm(3), s(0), s(0))
    nc.vector.tensor_mul(s(22), tm(3), s(2))
    nc.vector.tensor_scalar_mul(s(23), s(14), -4096.0)
    nc.vector.tensor_sub(tm(4), s(23), s(15))
    nc.vector.tensor_sub(s(12), tm(4), s(22))
    nc.vector.tensor_sub(tm(5), s(16), s(15))
    nc.vector.tensor_scalar_mul(tm(5), tm(5), 0.5)
    nc.vector.tensor_add(tm(6), s(23), tm(5))
    nc.vector.tensor_scalar_mul(tm(7), s(22), -0.5)
    nc.vector.tensor_add(s(19), tm(6), tm(7))

    def sb(i, np_=P, p0=0):
        return sv[p0 : p0 + np_, i : i + 1]

    # ---------- per-candidate coefficient vectors [P, U] ----------
    nf = pers.tile([P, U], F32)
    nc.vector.tensor_scalar_add(nf[:], gt[:], 1.0)
    zL = pers.tile([P, U], F32)
    nc.vector.tensor_scalar(zL[:], nf[:], sb(1), sb(2), ALU.mult, ALU.add)
    pvnL = pers.tile([P, U], F32)
    nc.vector.reciprocal(pvnL[:], zL[:])
    zR = pers.tile([P, U], F32)
    nc.vector.tensor_scalar(zR[:], gt[:], sb(3), sb(4), ALU.mult, ALU.add)
    pvnR = pers.tile([P, U], F32)
    nc.vector.reciprocal(pvnR[:], zR[:])
    lpvnL = pers.tile([P, U], F32)
    nc.scalar.activation(lpvnL[:], pvnL[:], AF.Ln)
    lpvnR = pers.tile([P, U], F32)
    nc.scalar.activation(lpvnR[:], pvnR[:], AF.Ln)
    kc2 = pers.tile([P, U], F32)
    nc.vector.tensor_add(kc2[:], lpvnL[:], lpvnR[:])

    nRf = pers.tile([P, U], F32)
    nc.vector.tensor_scalar(nRf[:], gt[:], -1.0, 8191.0, ALU.mult, ALU.add)
    gc = pers.tile([P, U], F32)
    nc.vector.tensor_scalar_max(gc[:], gt[:], 1.0)
    inv_n1 = pers.tile([P, U], F32)
    nc.vector.reciprocal(inv_n1[:], gc[:])
    nR1c = pers.tile([P, U], F32)
    nc.vector.tensor_scalar(nR1c[:], gt[:], -1.0, 8190.0, ALU.mult, ALU.add)
    nc.vector.tensor_scalar_max(nR1c[:], nR1c[:], 1.0)
    inv_nR1 = pers.tile([P, U], F32)
    nc.vector.reciprocal(inv_nR1[:], nR1c[:])
    inv_n = pers.tile([P, U], F32)
    nc.vector.reciprocal(inv_n[:], nf[:])
    inv_nR = pers.tile([P, U], F32)
    nRc = pers.tile([P, U], F32)
    nc.vector.tensor_scalar_max(nRc[:], nRf[:], 1.0)
    nc.vector.reciprocal(inv_nR[:], nRc[:])

    n_n1 = pers.tile([P, U], F32)
    nc.vector.tensor_mul(n_n1[:], nf[:], inv_n1[:])
    nR_nR1 = pers.tile([P, U], F32)
    nc.vector.tensor_mul(nR_nR1[:], nRf[:], inv_nR1[:])
    i_nn1 = pers.tile([P, U], F32)
    nc.vector.tensor_mul(i_nn1[:], inv_n[:], inv_n1[:])
    i_nRnR1 = pers.tile([P, U], F32)
    nc.vector.tensor_mul(i_nRnR1[:], inv_nR[:], inv_nR1[:])

    CBL = pers.tile([P, U], F32)
    nc.scalar.activation(CBL[:], n_n1[:], AF.Copy, scale=sb(7))
    CBR = pers.tile([P, U], F32)
    nc.scalar.activation(CBR[:], nR_nR1[:], AF.Copy, scale=sb(7))
    # CA2L = 0.5*kq*i_nn1 + 0.5*k^2*pvnL
    CA2L = pers.tile([P, U], F32)
    q1 = pers.tile([P, U], F32)
    nc.scalar.activation(q1[:], pvnL[:], AF.Copy, scale=sb(8))
    q2 = pers.tile([P, U], F32)
    nc.scalar.activation(q2[:], i_nn1[:], AF.Copy, scale=sb(9))
    nc.vector.tensor_add(CA2L[:], q1[:], q2[:])
    CA2R = pers.tile([P, U], F32)
    q1b = pers.tile([P, U], F32)
    nc.scalar.activation(q1b[:], pvnR[:], AF.Copy, scale=sb(8))
    q2b = pers.tile([P, U], F32)
    nc.scalar.activation(q2b[:], i_nRnR1[:], AF.Copy, scale=sb(9))
    nc.vector.tensor_add(CA2R[:], q1b[:], q2b[:])
    CAL = pers.tile([P, U], F32)
    nc.scalar.activation(CAL[:], pvnL[:], AF.Copy, scale=sb(10))
    CAR = pers.tile([P, U], F32)
    nc.scalar.activation(CAR[:], pvnR[:], AF.Copy, scale=sb(10))
    Cc = pers.tile([P, U], F32)
    p12 = pers.tile([P, U], F32)
    nc.vector.tensor_add(p12[:], pvnL[:], pvnR[:])
    cc1 = pers.tile([P, U], F32)
    nc.scalar.activation(cc1[:], p12[:], AF.Copy, scale=sb(11))
    cct = pers.tile([P, U], F32)
    nc.vector.tensor_scalar(cct[:], kc2[:], 0.5, sb(12), ALU.mult, ALU.add)
    nc.vector.tensor_add(Cc[:], cc1[:], cct[:])
    # bake the invalid-candidate mask into Cc: bf = ... + Cc ~ -1e30 there.
    # valid g in [15, 8175); compute via two is_ge comparisons on gt.
    mlo = pers.tile([P, U], F32)
    nc.vector.tensor_scalar(mlo[:], gt[:], 14.5, NEG, ALU.is_lt, ALU.mult)
    mhi = pers.tile([P, U], F32)
    nc.vector.tensor_scalar(mhi[:], gt[:], 8174.5, NEG, ALU.is_ge, ALU.mult)
    nc.vector.tensor_add(Cc[:], Cc[:], mlo[:])
    nc.vector.tensor_add(Cc[:], Cc[:], mhi[:])
    # near-end 0/1 mask (g >= 6553)
    nemask = pers.tile([P, U], F32)
    nc.vector.tensor_scalar(nemask[:], gt[:], 6552.5, None, ALU.is_ge)

    # ---------- persistent accumulators ----------
    bund = pers.tile([P, 80], F32)  # [0:16) rmax | [16:32) Zp | [32:48) En | [48:64) At | [64:80) Bt
    zeros = pers.tile([P, max(CHUNK_SIZES) * U], F32)
    nc.gpsimd.memset(zeros[:], 0.0)

    # ---------- finale tiles (filled incrementally from the last chunk) ----------
    Tall = ps1.tile([BL, 5, P], F32)
    Tm = Tall[:, 0, :]
    Tz = Tall[:, 1, :]
    Te = Tall[:, 2, :]
    Ta = Tall[:, 3, :]
    Tb = Tall[:, 4, :]
    M16 = pers.tile([BL, 1], F32)
    d = pers.tile([BL, P], F32)
    w = pers.tile([BL, P], F32)
    bfW = pers.tile([BL, 1], F32)
    sig = pers.tile([BL, 1], F32)

    def trace_bfw():
        # whole-window log marginal: only needs At/Bt (bund cols 48:80)
        nc.tensor.transpose(Ta, bund[:, 48 : 48 + BL], id_t[:])
        nc.tensor.transpose(Tb, bund[:, 64 : 64 + BL], id_t[:])
        At16 = Ta[:, 0:1]
        Bt16 = Tb[:, 0:1]
        t1 = pers.tile([BL, 1], F32)
        nc.scalar.activation(t1[:], At16, AF.Square, scale=1.0 / 32.0)
        v2 = pers.tile([BL, 1], F32)
        nc.vector.tensor_scalar_mul(v2[:], t1[:], 1.0 / 8192.0)
        vW = pers.tile([BL, 1], F32)
        nc.vector.scalar_tensor_tensor(
            vW[:], Bt16, 1.0 / 1024.0, v2[:], ALU.mult, ALU.subtract
        )
        nc.vector.tensor_scalar(vW[:], vW[:], 1.0 / 8191.0, 1.0e-8, ALU.mult, ALU.max)
        term1 = pers.tile([BL, 1], F32)
        nc.vector.tensor_scalar_mul(term1[:], vW[:], sb(17, BL))
        term2 = pers.tile([BL, 1], F32)
        nc.vector.tensor_scalar_mul(term2[:], t1[:], sb(18, BL))
        uu = pers.tile([BL, 1], F32)
        nc.scalar.activation(uu[:], At16, AF.Identity, bias=sb(6, BL), scale=sb(5, BL))
        u2 = pers.tile([BL, 1], F32)
        nc.scalar.activation(u2[:], uu[:], AF.Square)
        term3 = pers.tile([BL, 1], F32)
        nc.vector.tensor_scalar_mul(term3[:], u2[:], sb(13, BL))
        tsum = pers.tile([BL, 1], F32)
        nc.vector.tensor_add(tsum[:], term1[:], term2[:])
        nc.vector.tensor_sub(tsum[:], tsum[:], term3[:])
        nc.vector.tensor_scalar(bfW[:], tsum[:], -0.5, sb(19, BL), ALU.mult, ALU.add)

    def trace_maxw():
        # bund[:,0:16] holds -rmax; M16 = max(rmax) = -min(-rmax) via negate
        nc.tensor.transpose(Tm, bund[:, 0:BL], id_t[:])
        nc.vector.tensor_reduce(M16[:], Tm, AX.X, ALU.min, negate=True)
        # d = rmax_p - M16 = (-Tm) - M16
        nc.vector.tensor_scalar(d[:], Tm, -1.0, M16[:], ALU.mult, ALU.subtract)
        nc.scalar.activation(w[:], d[:], AF.Exp)
        sigin = pers.tile([BL, 1], F32)
        nc.vector.tensor_sub(sigin[:], M16[:], bfW[:])
        nc.scalar.activation(sig[:], sigin[:], AF.Sigmoid)

    # ---------- per-chunk pipeline ----------
    # trace each chunk's gpsimd fold ahead of the previous chunk's Pool work
    # (in-order engine streams: otherwise the fold queues behind the adds)
    xhs = [None] * len(chunks)

    def ufof(ci):
        return UF_SCHED[ci] if UF_SCHED is not None else UF

    def trace_fold(ci):
        uf = ufof(ci)
        bc = chunks[ci][1]
        if uf > 0:
            xh = wk.tile([P, bc, uf, 16], F32, tag="xh")
            nc.gpsimd.tensor_add(
                xh[:], xts[ci][:, :, 0:uf, 0:16], xts[ci][:, :, 0:uf, 16:32]
            )
            xhs[ci] = xh

    trace_fold(0)
    for ci, (bs, bc) in enumerate(chunks):
        xt = xts[ci]
        last = ci == len(chunks) - 1
        if not last:
            trace_fold(ci + 1)

        uf = ufof(ci)
        sr = wk.tile([P, bc, U], F32)
        if uf > 0:
            if ci == 0 and uf == 16:
                # quartered first chunk: reduce each raw quarter as it lands
                for q in range(1, 4):
                    nc.vector.tensor_reduce(
                        sr[:, :, q * 16 : (q + 1) * 16],
                        xt[:, :, q * 16 : (q + 1) * 16, :],
                        AX.X,
                        ALU.add,
                    )
            elif uf < U:
                nc.vector.tensor_reduce(
                    sr[:, :, uf:U], xt[:, :, uf:U, :], AX.X, ALU.add
                )
            nc.vector.tensor_reduce(sr[:, :, 0:uf], xhs[ci][:], AX.X, ALU.add)
        else:
            HU = U // 2
            nc.vector.tensor_reduce(
                sr[:, :, 0:HU], xt[:, :, 0:HU, :], AX.X, ALU.add
            )
            nc.vector.tensor_reduce(
                sr[:, :, HU:U], xt[:, :, HU:U, :], AX.X, ALU.add
            )
        sq = wk.tile([P, bc, U], F32)
        nc.scalar.activation(sq[:], sr[:], AF.Square)

        A = wk.tile([P, bc, U], F32)
        nc.vector.tensor_tensor_scan(
            A[:].rearrange("p b u -> p (b u)"),
            sr[:].rearrange("p b u -> p (b u)"),
            zeros[:, 0 : bc * U],
            0.0,
            ALU.add,
            ALU.add,
        )
        Bt_ = wk.tile([P, bc, U], F32)
        nc.vector.tensor_tensor_scan(
            Bt_[:].rearrange("p b u -> p (b u)"),
            sq[:].rearrange("p b u -> p (b u)"),
            zeros[:, 0 : bc * U],
            0.0,
            ALU.add,
            ALU.add,
        )

        # carry fix: rowprev, chunk totals, triangular matmul
        rv = wk.tile([P, 2 * bc], F32)  # [0:bc) rvA | [bc:2BC) rvB
        nc.gpsimd.memset(rv[:, 0:1], 0.0)
        nc.gpsimd.memset(rv[:, bc : bc + 1], 0.0)
        nc.vector.tensor_copy(rv[:, 1:bc], A[:, 0 : bc - 1, U - 1])
        nc.vector.tensor_copy(rv[:, bc + 1 : 2 * bc], Bt_[:, 0 : bc - 1, U - 1])
        ct = wk.tile([P, 2 * bc], F32)
        nc.vector.tensor_sub(ct[:, 0:bc], A[:, :, U - 1], rv[:, 0:bc])
        nc.vector.tensor_sub(ct[:, bc : 2 * bc], Bt_[:, :, U - 1], rv[:, bc : 2 * bc])
        g_ps = psp.tile([P, 2 * bc], F32)
        nc.tensor.matmul(g_ps[:], ut_t[:], ct[:])
        tot_ps = psp.tile([P, 2 * bc], F32)
        nc.tensor.matmul(tot_ps[:], ones_t[:], ct[:])
        off = wk.tile([P, 2 * bc], F32)
        nc.vector.tensor_sub(off[:], g_ps[:], rv[:])

        offA_b = off[:, 0:bc].unsqueeze(2).broadcast_to([P, bc, U])
        offB_b = off[:, bc : 2 * bc].unsqueeze(2).broadcast_to([P, bc, U])
        nc.vector.tensor_add(A[:], A[:], offA_b)
        nc.vector.tensor_add(Bt_[:], Bt_[:], offB_b)

        # At/Bt to SBUF (bund doubles as the staging buffer; gpsimd can't read PSUM)
        nc.scalar.copy(bund[:, 48 + bs : 48 + bs + bc], tot_ps[:, 0:bc])
        nc.scalar.copy(bund[:, 64 + bs : 64 + bs + bc], tot_ps[:, bc : 2 * bc])
        if last:
            trace_bfw()
        At_b = (
            bund[:, 48 + bs : 48 + bs + bc].unsqueeze(2).broadcast_to([P, bc, U])
        )
        Btot_b = (
            bund[:, 64 + bs : 64 + bs + bc].unsqueeze(2).broadcast_to([P, bc, U])
        )
        AR = wk.tile([P, bc, U], F32)
        nc.gpsimd.tensor_sub(AR[:], At_b, A[:])
        BR = wk.tile([P, bc, U], F32)
        nc.gpsimd.tensor_sub(BR[:], Btot_b, Bt_[:])

        A2 = wk.tile([P, bc, U], F32)
        nc.scalar.activation(A2[:], A[:], AF.Square)
        AR2 = wk.tile([P, bc, U], F32)
        nc.scalar.activation(AR2[:], AR[:], AF.Square)

        def cb(t):
            return t[:].unsqueeze(1).broadcast_to([P, bc, U])

        bf = wk.tile([P, bc, U], F32)
        p2 = wk.tile([P, bc, U], F32)
        p3 = wk.tile([P, bc, U], F32)
        p5 = wk.tile([P, bc, U], F32)
        p6 = wk.tile([P, bc, U], F32)
        nc.vector.tensor_mul(p2[:], A2[:], cb(CA2L))
        nc.vector.tensor_mul(p3[:], Bt_[:], cb(CBL))
        nc.vector.tensor_mul(p5[:], AR2[:], cb(CA2R))
        (nc.vector if last else nc.gpsimd).tensor_mul(p6[:], BR[:], cb(CBR))
        if pm_zero:
            # c = pm/pv = 0 -> the A and AR linear terms vanish
            eng1 = nc.vector if last else nc.gpsimd
            eng1.tensor_add(p2[:], p2[:], p3[:])
            nc.vector.tensor_add(p5[:], p5[:], p6[:])
            eng1.tensor_add(p2[:], p2[:], cb(Cc))
            nc.vector.tensor_add(bf[:], p2[:], p5[:])
        else:
            p1 = wk.tile([P, bc, U], F32)
            p4 = wk.tile([P, bc, U], F32)
            nc.vector.tensor_mul(p1[:], A[:], cb(CAL))
            nc.vector.tensor_mul(p4[:], AR[:], cb(CAR))
            nc.gpsimd.tensor_add(p1[:], p1[:], p2[:])
            nc.vector.tensor_add(p3[:], p3[:], p4[:])
            nc.gpsimd.tensor_add(p5[:], p5[:], p6[:])
            nc.gpsimd.tensor_add(p1[:], p1[:], cb(Cc))
            nc.vector.tensor_add(p3[:], p3[:], p5[:])
            nc.vector.tensor_add(bf[:], p1[:], p3[:])

        # per-(p,b) NEGATED max (negate=True -> -max), exp with shift, partial sums
        nc.vector.tensor_reduce(
            bund[:, bs : bs + bc], bf[:], AX.X, ALU.max, negate=True
        )
        if last:
            trace_maxw()
        e = wk.tile([P, bc, U], F32)
        for b in range(bc):
            nc.scalar.activation(
                e[:, b, :],
                bf[:, b, :],
                AF.Exp,
                bias=bund[:, bs + b : bs + b + 1],
                accum_out=bund[:, 16 + bs + b : 17 + bs + b],
            )
        # near-end partial sums: sum_u e * nemask
        en = wk.tile([P, bc, U], F32)
        (nc.vector if last else nc.gpsimd).tensor_mul(en[:], e[:], cb(nemask))
        nc.vector.tensor_reduce(
            bund[:, 32 + bs : 32 + bs + bc], en[:], AX.X, ALU.add
        )

    # ---------- finale: Zb/Nb combine (Tm/Ta/Tb + bfW already traced) ----------
    nc.tensor.transpose(Tz, bund[:, 16 : 16 + BL], id_t[:])
    nc.tensor.transpose(Te, bund[:, 32 : 32 + BL], id_t[:])
    wz = pers.tile([BL, P], F32)
    Zb = pers.tile([BL, 1], F32)
    nc.vector.scalar_tensor_tensor(
        wz[:], w[:], 1.0, Tz, ALU.mult, ALU.mult, accum_out=Zb[:]
    )
    wn = pers.tile([BL, P], F32)
    Nb = pers.tile([BL, 1], F32)
    nc.vector.scalar_tensor_tensor(
        wn[:], w[:], 1.0, Te, ALU.mult, ALU.mult, accum_out=Nb[:]
    )
    invZ = pers.tile([BL, 1], F32)
    nc.vector.reciprocal(invZ[:], Zb[:])
    ratio = pers.tile([BL, 1], F32)
    nc.vector.tensor_mul(ratio[:], Nb[:], invZ[:])
    outv = pers.tile([BL, 1], F32)
    nc.vector.tensor_mul(outv[:], sig[:], ratio[:])
    nc.sync.dma_start(out[:], outv[:])


def host_consts():
    ut = np.triu(np.ones((P, P), np.float32), 1)
    ident = np.eye(P, dtype=np.float32)
    return ut, ident


def split_multi_waits(nc):
    """Walrus in this container allows one sync wait per instruction; move
    extra waits onto same-engine NOPs inserted immediately before."""
    import bass_rust

    nid = [0]
    for f in nc.m.functions:
        for b in f.blocks:
            insts = b.instructions
            i = 0
            while i < len(insts):
                ins = insts[i]
                si = ins.sync_info
                if si is not None and si.on_wait is not None and len(si.on_wait) > 1:
                    waits = list(si.on_wait)
                    for w in waits[:-1]:
                        nop = mybir.InstNoOp(
                            name=f"I-waitsplit-{nid[0]}", ins=[], outs=[]
                        )
                        nid[0] += 1
                        nop.engine = ins.engine
                        nop.sync_info = bass_rust.SyncInfo(
                            on_wait=[w], on_update=[]
                        )
                        insts.insert(i, nop)
                        i += 1
                    si.on_wait = waits[-1:]
                i += 1


_NC_CACHE = {}


def build_nc(split=True, reps=1, pm_zero=False):
    global _NC_CACHE
    key = (split, reps, pm_zero)
    if key in _NC_CACHE:
        return _NC_CACHE[key]
    nc = bass.Bass()
    x = nc.declare_dram_parameter("x", [BL, T, N], F32, isOutput=False)
    params = nc.declare_dram_parameter("params", [P, 3], F32, isOutput=False)
    utc = nc.declare_dram_parameter("utc", [P, P], F32, isOutput=False)
    idc = nc.declare_dram_parameter("idc", [P, P], F32, isOutput=False)
    out = nc.declare_dram_parameter("out", [BL, 1], F32, isOutput=True)
    with tile.TileContext(nc) as tc:
        for _ in range(reps):
            with ExitStack() as ctx:
                build_body(
                    ctx, tc, x[:], params[:], utc[:], idc[:],
                    out[:], pm_zero=pm_zero,
                )
    if split:
        split_multi_waits(nc)
    _NC_CACHE[key] = nc
    return nc


def make_in_maps(x, prior_mean, prior_var, noise_var):
    x = np.ascontiguousarray(np.asarray(x, dtype=np.float32))
    params = np.tile(
        np.array(
            [[float(prior_mean[0]), float(prior_var[0]), float(noise_var[0])]],
            dtype=np.float32,
        ),
        (P, 1),
    )
    ut, ident = host_consts()
    in_maps = []
    for c in range(NCORES):
        in_maps.append(
            {
                "x": x[c * BL : (c + 1) * BL],
                "params": params,
                "utc": ut,
                "idc": ident,
            }
        )
    return in_maps


def kernel(x, prior_mean, prior_var, noise_var):
    from concourse.bass_utils import run_bass_kernel_spmd

    in_maps = make_in_maps(x, prior_mean, prior_var, noise_var)
    nc = build_nc(pm_zero=(float(np.asarray(prior_mean).reshape(-1)[0]) == 0.0))
    res = run_bass_kernel_spmd(nc, in_maps, list(range(NCORES)))
    outs = [np.asarray(res.results[c]["out"]).reshape(BL) for c in range(NCORES)]
    return np.concatenate(outs).astype(np.float32)



# revision 8
# speedup vs baseline: 1.2408x; 1.2408x over previous
"""Trainium2 Bass kernel for BayesianChangePointDetector (segment_reduce).

Contract: kernel(**inputs) takes FULL inputs (x:[128,8192,32] f32, plus 3
scalar prior params) and returns the FULL [128] f32 output. Internally the
batch dim is sharded across 8 NeuronCores (16 rows each, pure data parallel,
no collectives), each core runs the same Bass/Tile program, and the host
concatenates the 8 per-core [16] outputs.

Fast path (pm == 0, the shipped input): x is cast f32->fp16 during the SWDGE
DMA (halves the charged DMA time), the N=32 fold is a fp16 binary add-tree
(32->16->8->4 on DVE in 2x perf mode, 4->2->1 on Pool), prefix sums are
per-row f32 scans with a triangular-ones carry matmul on PE, and the Bayes
factor is assembled as bf'' = CA2L*A^2 + CA2R*(At-A)^2 + SB*B + CBR2*Btot[b]
+ Cc_var with all large per-row constants moved into the final sigmoid path,
so bf'' is O(10) and exp needs no max-shift. A^2 and (At-A)^2 come from ACT
Square activations whose bias folds in the carry offsets.
"""

import sys

if "/opt/trn_rl_repo" not in sys.path:
    sys.path.insert(0, "/opt/trn_rl_repo")

import math
from contextlib import ExitStack

import numpy as np

import concourse.bass as bass
import concourse.tile as tile
from concourse import mybir

F32 = mybir.dt.float32
F16 = mybir.dt.float16
AF = mybir.ActivationFunctionType
ALU = mybir.AluOpType
AX = mybir.AxisListType

B, T, N = 128, 8192, 32
NCORES = 8
BL = B // NCORES  # 16 batch rows per core
P = 128           # partitions = t-blocks
U = T // P        # 64 t's per partition
NS = 32           # scalar-slot count
NEG = -1.0e30

# fast-path chunking: small last chunk to shrink the post-DMA tail
CHUNKS_PM0 = [(0, 4), (4, 5), (9, 5), (14, 2)]

# near-end threshold: candidates g >= 6553 (g = 64p+u); p=102 partial (u>=25),
# p>=103 fully near-end.
NE_P = 102
NE_U = 25
# valid candidates: g in [15, 8175)
LO_THR = 14.5
HI_THR = 8174.5


def scalar_prep(nc, pers, ptile):
    """Per-partition scalar slots, identical math to the baseline kernel."""
    sv = pers.tile([P, NS], F32)
    tmp = pers.tile([P, 8], F32)

    def s(i):
        return sv[:, i : i + 1]

    def tm(i):
        return tmp[:, i : i + 1]

    # softplus(x) = ln(1 + exp(x))
    nc.scalar.activation(tm(0), ptile[:, 1:2], AF.Exp)
    nc.vector.tensor_scalar_add(tm(0), tm(0), 1.0)
    nc.scalar.activation(s(20), tm(0), AF.Ln)
    nc.scalar.activation(tm(1), ptile[:, 2:3], AF.Exp)
    nc.vector.tensor_scalar_add(tm(1), tm(1), 1.0)
    nc.scalar.activation(s(21), tm(1), AF.Ln)
    nc.vector.tensor_copy(s(0), ptile[:, 0:1])
    nc.vector.reciprocal(s(1), s(21))
    nc.vector.reciprocal(s(2), s(20))
    nc.vector.tensor_scalar_mul(s(3), s(1), -1.0)
    nc.vector.tensor_scalar(s(4), s(1), 8191.0, s(2), ALU.mult, ALU.add)
    nc.vector.tensor_scalar_mul(s(5), s(1), 1.0 / 32.0)
    nc.vector.tensor_mul(s(6), s(0), s(2))
    nc.vector.tensor_scalar_mul(s(7), s(1), -0.5 / 1024.0)
    nc.vector.tensor_scalar_mul(s(9), s(1), 0.5 / 1024.0)
    nc.vector.tensor_mul(tm(0), s(5), s(5))
    nc.vector.tensor_scalar_mul(s(8), tm(0), 0.5)
    nc.vector.tensor_mul(s(10), s(6), s(5))
    nc.vector.tensor_mul(tm(1), s(6), s(6))
    nc.vector.tensor_scalar_mul(s(11), tm(1), 0.5)
    nc.scalar.activation(s(14), s(21), AF.Ln, scale=2.0 * math.pi)
    nc.scalar.activation(s(15), s(20), AF.Ln)
    nc.vector.tensor_scalar_mul(s(17), s(1), 8192.0)
    nc.vector.tensor_scalar(tm(2), s(1), 8192.0, s(2), ALU.mult, ALU.add)
    nc.vector.reciprocal(s(13), tm(2))
    nc.scalar.activation(s(16), s(13), AF.Ln)
    nc.vector.tensor_scalar_mul(s(18), s(1), 1.0 / 8192.0)
    nc.vector.tensor_mul(tm(3), s(0), s(0))
    nc.vector.tensor_mul(s(22), tm(3), s(2))
    nc.vector.tensor_scalar_mul(s(23), s(14), -4096.0)
    nc.vector.tensor_sub(tm(4), s(23), s(15))
    nc.vector.tensor_sub(s(12), tm(4), s(22))
    nc.vector.tensor_sub(tm(5), s(16), s(15))
    nc.vector.tensor_scalar_mul(tm(5), tm(5), 0.5)
    nc.vector.tensor_add(tm(6), s(23), tm(5))
    nc.vector.tensor_scalar_mul(tm(7), s(22), -0.5)
    nc.vector.tensor_add(s(19), tm(6), tm(7))
    return sv


def build_body_pm0(ctx, tc, x, params, utc, idc, out):
    nc = tc.nc
    pers = ctx.enter_context(tc.tile_pool(name="pers", bufs=1))
    xp = ctx.enter_context(tc.tile_pool(name="xp", bufs=2))
    wk = ctx.enter_context(tc.tile_pool(name="wk", bufs=2))
    psp = ctx.enter_context(tc.tile_pool(name="psp", bufs=2, space="PSUM"))
    ps1 = ctx.enter_context(tc.tile_pool(name="ps1", bufs=1, space="PSUM"))

    chunks = CHUNKS_PM0
    nch = len(chunks)

    # ---------- consts + early DMAs ----------
    ut_t = pers.tile([P, P], F32)
    ones_t = pers.tile([P, P], F32)
    id_t = pers.tile([P, P], F32)
    gt = pers.tile([P, U], F32)
    ptile = pers.tile([P, 3], F32)
    bund = pers.tile([P, 80], F32)  # [0:16) -rmax | [16:32) Zp | [32:48) En | [48:64) At | [64:80) Btot
    zeros = pers.tile([P, U], F32)

    nc.sync.dma_start(ptile[:], params[:])

    xts = [None] * nch

    def issue_dma(ci):
        bs, bc = chunks[ci]
        xt = xp.tile([P, bc, U, N], F16, tag="xt")
        src = x[bs : bs + bc].rearrange("b (p u) n -> p b u n", p=P)
        if ci == 0:
            HU = U // 2
            nc.gpsimd.dma_start(xt[:, :, 0:HU, :], src[:, :, 0:HU, :])
            nc.gpsimd.dma_start(xt[:, :, HU:U, :], src[:, :, HU:U, :])
        else:
            nc.gpsimd.dma_start(xt[:], src[:])
        xts[ci] = xt

    # Pool stream: iota, chunk0 + chunk1 DMA descriptors, then memsets
    gti = pers.tile([P, U], mybir.dt.int32)
    nc.gpsimd.iota(gti[:], [[1, U]], base=0, channel_multiplier=U)
    issue_dma(0)
    issue_dma(1)
    nc.gpsimd.memset(ones_t[:], 1.0)
    nc.gpsimd.memset(zeros[:], 0.0)

    # HWDGE consts behind params
    nc.sync.dma_start(ut_t[:], utc[:])
    nc.sync.dma_start(id_t[:], idc[:])

    nc.vector.tensor_copy(gt[:], gti[:])
    sv = scalar_prep(nc, pers, ptile)

    def sb(i, np_=P, p0=0):
        return sv[p0 : p0 + np_, i : i + 1]

    # ---------- per-candidate coefficient vectors [P, U] ----------
    nf = pers.tile([P, U], F32)
    nc.vector.tensor_scalar_add(nf[:], gt[:], 1.0)
    zL = pers.tile([P, U], F32)
    nc.vector.tensor_scalar(zL[:], nf[:], sb(1), sb(2), ALU.mult, ALU.add)
    pvnL = pers.tile([P, U], F32)
    nc.vector.reciprocal(pvnL[:], zL[:])
    zR = pers.tile([P, U], F32)
    nc.vector.tensor_scalar(zR[:], gt[:], sb(3), sb(4), ALU.mult, ALU.add)
    pvnR = pers.tile([P, U], F32)
    nc.vector.reciprocal(pvnR[:], zR[:])
    lpvnL = pers.tile([P, U], F32)
    nc.scalar.activation(lpvnL[:], pvnL[:], AF.Ln)
    lpvnR = pers.tile([P, U], F32)
    nc.scalar.activation(lpvnR[:], pvnR[:], AF.Ln)
    # Cc_var = 0.5*(ln pvnL + ln pvnR) + validity masks (NO sc constant here)
    Ccv = pers.tile([P, U], F32)
    nc.vector.tensor_add(Ccv[:], lpvnL[:], lpvnR[:])
    nc.vector.tensor_scalar_mul(Ccv[:], Ccv[:], 0.5)
    mlo = pers.tile([P, U], F32)
    nc.vector.tensor_scalar(mlo[:], gt[:], LO_THR, NEG, ALU.is_lt, ALU.mult)
    mhi = pers.tile([P, U], F32)
    nc.vector.tensor_scalar(mhi[:], gt[:], HI_THR, NEG, ALU.is_ge, ALU.mult)
    nc.vector.tensor_add(Ccv[:], Ccv[:], mlo[:])
    nc.vector.tensor_add(Ccv[:], Ccv[:], mhi[:])

    gc = pers.tile([P, U], F32)
    nc.vector.tensor_scalar_max(gc[:], gt[:], 1.0)
    inv_n1 = pers.tile([P, U], F32)
    nc.vector.reciprocal(inv_n1[:], gc[:])
    nR1c = pers.tile([P, U], F32)
    nc.vector.tensor_scalar(nR1c[:], gt[:], -1.0, 8190.0, ALU.mult, ALU.add)
    nc.vector.tensor_scalar_max(nR1c[:], nR1c[:], 1.0)
    inv_nR1 = pers.tile([P, U], F32)
    nc.vector.reciprocal(inv_nR1[:], nR1c[:])
    inv_n = pers.tile([P, U], F32)
    nc.vector.reciprocal(inv_n[:], nf[:])
    nRf = pers.tile([P, U], F32)
    nc.vector.tensor_scalar(nRf[:], gt[:], -1.0, 8191.0, ALU.mult, ALU.add)
    nRc = pers.tile([P, U], F32)
    nc.vector.tensor_scalar_max(nRc[:], nRf[:], 1.0)
    inv_nR = pers.tile([P, U], F32)
    nc.vector.reciprocal(inv_nR[:], nRc[:])

    n_n1 = pers.tile([P, U], F32)
    nc.vector.tensor_mul(n_n1[:], nf[:], inv_n1[:])
    nR_nR1 = pers.tile([P, U], F32)
    nc.vector.tensor_mul(nR_nR1[:], nRf[:], inv_nR1[:])
    i_nn1 = pers.tile([P, U], F32)
    nc.vector.tensor_mul(i_nn1[:], inv_n[:], inv_n1[:])
    i_nRnR1 = pers.tile([P, U], F32)
    nc.vector.tensor_mul(i_nRnR1[:], inv_nR[:], inv_nR1[:])

    # CA2L = (k^2/2)*pvnL + (kq/2)*i_nn1 ; CA2R analog
    CA2L = pers.tile([P, U], F32)
    q1 = pers.tile([P, U], F32)
    nc.scalar.activation(q1[:], pvnL[:], AF.Copy, scale=sb(8))
    q2 = pers.tile([P, U], F32)
    nc.scalar.activation(q2[:], i_nn1[:], AF.Copy, scale=sb(9))
    nc.vector.tensor_add(CA2L[:], q1[:], q2[:])
    CA2R = pers.tile([P, U], F32)
    q1b = pers.tile([P, U], F32)
    nc.scalar.activation(q1b[:], pvnR[:], AF.Copy, scale=sb(8))
    q2b = pers.tile([P, U], F32)
    nc.scalar.activation(q2b[:], i_nRnR1[:], AF.Copy, scale=sb(9))
    nc.vector.tensor_add(CA2R[:], q1b[:], q2b[:])
    # SB = CBL - CBR = (-kq/2)*(n/(n-1) - nR/(nR-1))
    dsb = pers.tile([P, U], F32)
    nc.vector.tensor_sub(dsb[:], n_n1[:], nR_nR1[:])
    SBt = pers.tile([P, U], F32)
    nc.scalar.activation(SBt[:], dsb[:], AF.Copy, scale=sb(7))
    # CBR2 = (-kq/2) * 1/(nR-1)
    CBR2 = pers.tile([P, U], F32)
    nc.scalar.activation(CBR2[:], inv_nR1[:], AF.Copy, scale=sb(7))

    # ---------- finale tiles ----------
    Tall = ps1.tile([BL, 5, P], F32)
    Tm = Tall[:, 0, :]
    Tz = Tall[:, 1, :]
    Te = Tall[:, 2, :]
    Ta = Tall[:, 3, :]
    Tb = Tall[:, 4, :]
    M16 = pers.tile([BL, 1], F32)
    bfW = pers.tile([BL, 1], F32)
    sig = pers.tile([BL, 1], F32)

    def cbc(t, bc):
        return t[:].unsqueeze(1).broadcast_to([P, bc, U])

    # ---------- per-chunk pipeline ----------
    for ci, (bs, bc) in enumerate(chunks):
        xt = xts[ci]
        last = ci == nch - 1

        # DVE fold tree (fp16, 2x mode)
        h1 = wk.tile([P, bc, U, 16], F16, tag="h1")
        if ci == 0:
            HU = U // 2
            nc.vector.tensor_add(
                h1[:, :, 0:HU, :], xt[:, :, 0:HU, 0:16], xt[:, :, 0:HU, 16:32]
            )
            nc.vector.tensor_add(
                h1[:, :, HU:U, :], xt[:, :, HU:U, 0:16], xt[:, :, HU:U, 16:32]
            )
        else:
            nc.vector.tensor_add(h1[:], xt[:, :, :, 0:16], xt[:, :, :, 16:32])
        h2 = wk.tile([P, bc, U, 8], F16, tag="h2")
        nc.vector.tensor_add(h2[:], h1[:, :, :, 0:8], h1[:, :, :, 8:16])
        h3 = wk.tile([P, bc, U, 4], F16, tag="h3")
        nc.vector.tensor_add(h3[:], h2[:, :, :, 0:4], h2[:, :, :, 4:8])
        # Pool folds
        h4 = wk.tile([P, bc, U, 2], F16, tag="h4")
        nc.gpsimd.tensor_add(h4[:], h3[:, :, :, 0:2], h3[:, :, :, 2:4])
        sr = wk.tile([P, bc, U], F32, tag="sr")
        nc.gpsimd.tensor_add(sr[:], h4[:, :, :, 0:1], h4[:, :, :, 1:2])

        # next-next chunk DMA descriptors land here in the Pool stream
        if ci + 2 < nch:
            issue_dma(ci + 2)

        sq = wk.tile([P, bc, U], F32, tag="sq")
        nc.scalar.activation(sq[:], sr[:], AF.Square)

        A = wk.tile([P, bc, U], F32, tag="A")
        Bt_ = wk.tile([P, bc, U], F32, tag="B")
        for b in range(bc):
            nc.vector.tensor_tensor_scan(
                A[:, b, :], sr[:, b, :], zeros[:], 0.0, ALU.add, ALU.add
            )
        for b in range(bc):
            nc.vector.tensor_tensor_scan(
                Bt_[:, b, :], sq[:, b, :], zeros[:], 0.0, ALU.add, ALU.add
            )

        # carry fix: per-row totals -> triangular matmul
        ct = wk.tile([P, 2 * bc], F32, tag="ct")
        nc.scalar.copy(ct[:, 0:bc], A[:, :, U - 1])
        nc.scalar.copy(ct[:, bc : 2 * bc], Bt_[:, :, U - 1])
        g_ps = psp.tile([P, 2 * bc], F32, tag="gps")
        nc.tensor.matmul(g_ps[:], ut_t[:], ct[:])
        tot_ps = psp.tile([P, 2 * bc], F32, tag="tot")
        nc.tensor.matmul(tot_ps[:], ones_t[:], ct[:])

        offs = wk.tile([P, 2 * bc], F32, tag="offs")
        nc.scalar.copy(offs[:], g_ps[:])
        nc.scalar.copy(bund[:, 48 + bs : 48 + bs + bc], tot_ps[:, 0:bc])
        nc.scalar.copy(bund[:, 64 + bs : 64 + bs + bc], tot_ps[:, bc : 2 * bc])
        atm = wk.tile([P, bc], F32, tag="atm")
        nc.vector.tensor_sub(atm[:], bund[:, 48 + bs : 48 + bs + bc], offs[:, 0:bc])

        # A2 = (A+offA)^2, AR2 = (At - (A+offA))^2 via ACT Square with bias
        A2 = wk.tile([P, bc, U], F32, tag="A2")
        AR2 = wk.tile([P, bc, U], F32, tag="AR2")
        for b in range(bc):
            nc.scalar.activation(
                A2[:, b, :], A[:, b, :], AF.Square, bias=offs[:, b : b + 1]
            )
            nc.scalar.activation(
                AR2[:, b, :], A[:, b, :], AF.Square,
                bias=atm[:, b : b + 1], scale=-1.0,
            )

        # B += offB (Pool)
        offB_b = offs[:, bc : 2 * bc].unsqueeze(2).broadcast_to([P, bc, U])
        nc.gpsimd.tensor_add(Bt_[:], Bt_[:], offB_b)

        # bf'' = CA2L*A2 + CA2R*AR2 + SB*B + (CBR2*Btot[b] + Cc_var)
        nc.vector.tensor_mul(A2[:], A2[:], cbc(CA2L, bc))
        nc.vector.tensor_mul(AR2[:], AR2[:], cbc(CA2R, bc))
        nc.gpsimd.tensor_mul(Bt_[:], Bt_[:], cbc(SBt, bc))
        Kc = wk.tile([P, bc, U], F32, tag="Kc")
        for b in range(bc):
            nc.vector.scalar_tensor_tensor(
                Kc[:, b, :], CBR2[:], bund[:, 64 + bs + b : 65 + bs + b],
                Ccv[:], ALU.mult, ALU.add,
            )
        nc.vector.tensor_add(A2[:], A2[:], AR2[:])
        nc.gpsimd.tensor_add(Bt_[:], Bt_[:], Kc[:])
        nc.vector.tensor_add(A2[:], A2[:], Bt_[:])  # A2 now holds bf''

        # -max(bf'') per (p,b) for the confidence path
        nc.vector.tensor_reduce(
            bund[:, bs : bs + bc], A2[:], AX.X, ALU.max, negate=True
        )

        # e = exp(bf'') raw, Zp accumulated per b
        e = wk.tile([P, bc, U], F32, tag="e")
        for b in range(bc):
            nc.scalar.activation(
                e[:, b, :], A2[:, b, :], AF.Exp,
                accum_out=bund[:, 16 + bs + b : 17 + bs + b],
            )
        # per-(p,b) tail sums over u >= NE_U; only row p=102 is used (the
        # finale takes full Zp rows for p >= 103 from the Tz transpose)
        nc.vector.tensor_reduce(
            bund[:, 32 + bs : 32 + bs + bc], e[:, :, NE_U:U], AX.X, ALU.add
        )

    # ---------- finale ----------
    # whole-window log marginal (needs At/Btot of all rows)
    nc.tensor.transpose(Ta, bund[:, 48 : 48 + BL], id_t[:])
    nc.tensor.transpose(Tb, bund[:, 64 : 64 + BL], id_t[:])
    At16 = Ta[:, 0:1]
    Bt16 = Tb[:, 0:1]
    t1 = pers.tile([BL, 1], F32)
    nc.scalar.activation(t1[:], At16, AF.Square, scale=1.0 / 32.0)
    v2 = pers.tile([BL, 1], F32)
    nc.vector.tensor_scalar_mul(v2[:], t1[:], 1.0 / 8192.0)
    vW = pers.tile([BL, 1], F32)
    nc.vector.scalar_tensor_tensor(
        vW[:], Bt16, 1.0 / 1024.0, v2[:], ALU.mult, ALU.subtract
    )
    nc.vector.tensor_scalar(vW[:], vW[:], 1.0 / 8191.0, 1.0e-8, ALU.mult, ALU.max)
    term1 = pers.tile([BL, 1], F32)
    nc.vector.tensor_scalar_mul(term1[:], vW[:], sb(17, BL))
    term2 = pers.tile([BL, 1], F32)
    nc.vector.tensor_scalar_mul(term2[:], t1[:], sb(18, BL))
    uu = pers.tile([BL, 1], F32)
    nc.scalar.activation(uu[:], At16, AF.Identity, bias=sb(6, BL), scale=sb(5, BL))
    u2 = pers.tile([BL, 1], F32)
    nc.scalar.activation(u2[:], uu[:], AF.Square)
    term3 = pers.tile([BL, 1], F32)
    nc.vector.tensor_scalar_mul(term3[:], u2[:], sb(13, BL))
    tsum = pers.tile([BL, 1], F32)
    nc.vector.tensor_add(tsum[:], term1[:], term2[:])
    nc.vector.tensor_sub(tsum[:], tsum[:], term3[:])
    nc.vector.tensor_scalar(bfW[:], tsum[:], -0.5, sb(19, BL), ALU.mult, ALU.add)
    # cW = sc - (kq/2)*Btot - bfW ; true max = M16 + cW
    cw1 = pers.tile([BL, 1], F32)
    nc.vector.scalar_tensor_tensor(
        cw1[:], Bt16, sb(9, BL), bfW[:], ALU.mult, ALU.add
    )
    cW = pers.tile([BL, 1], F32)
    nc.vector.tensor_scalar(cW[:], cw1[:], -1.0, sb(12, BL), ALU.mult, ALU.add)

    nc.tensor.transpose(Tm, bund[:, 0:BL], id_t[:])
    nc.vector.tensor_reduce(M16[:], Tm, AX.X, ALU.min, negate=True)
    sigin = pers.tile([BL, 1], F32)
    nc.vector.tensor_add(sigin[:], M16[:], cW[:])
    nc.scalar.activation(sig[:], sigin[:], AF.Sigmoid)

    nc.tensor.transpose(Tz, bund[:, 16 : 16 + BL], id_t[:])
    nc.tensor.transpose(Te, bund[:, 32 : 32 + BL], id_t[:])
    Zb = pers.tile([BL, 1], F32)
    nc.vector.tensor_reduce(Zb[:], Tz, AX.X, ALU.add)
    # near-end mass: full Zp rows p in [103,128) plus the p=102 tail sum
    Nbh = pers.tile([BL, 1], F32)
    nc.vector.tensor_reduce(Nbh[:], Tz[:, NE_P + 1 : P], AX.X, ALU.add)
    Nb = pers.tile([BL, 1], F32)
    nc.vector.tensor_add(Nb[:], Nbh[:], Te[:, NE_P : NE_P + 1])
    invZ = pers.tile([BL, 1], F32)
    nc.vector.reciprocal(invZ[:], Zb[:])
    ratio = pers.tile([BL, 1], F32)
    nc.vector.tensor_mul(ratio[:], Nb[:], invZ[:])
    outv = pers.tile([BL, 1], F32)
    nc.vector.tensor_mul(outv[:], sig[:], ratio[:])
    nc.sync.dma_start(out[:], outv[:])


# ---------------------------------------------------------------------------
# General path (pm != 0): the proven baseline implementation, f32 throughout.
# ---------------------------------------------------------------------------

BC = 4
NCHUNK = BL // BC
UF_SCHED = [16, 32, 32, 32]
CHUNK_SIZES = [4, 4, 4, 4]
XP_BUFS = 2
WK_BUFS = 3
NE_P0 = 6553 // U
NE_U0 = 6553 - NE_P0 * U


def build_body(ctx, tc, x, params, utc, idc, out, pm_zero=False):
    nc = tc.nc
    pers = ctx.enter_context(tc.tile_pool(name="pers", bufs=1))
    xp = ctx.enter_context(tc.tile_pool(name="xp", bufs=XP_BUFS))
    wk = ctx.enter_context(tc.tile_pool(name="wk", bufs=WK_BUFS))
    psp = ctx.enter_context(tc.tile_pool(name="psp", bufs=2, space="PSUM"))
    ps1 = ctx.enter_context(tc.tile_pool(name="ps1", bufs=1, space="PSUM"))

    ut_t = pers.tile([P, P], F32)
    ones_t = pers.tile([P, P], F32)
    id_t = pers.tile([P, P], F32)
    gt = pers.tile([P, U], F32)
    ptile = pers.tile([P, 3], F32)
    nc.sync.dma_start(ptile[:], params[:])
    nc.gpsimd.memset(ones_t[:], 1.0)
    gti = pers.tile([P, U], mybir.dt.int32)
    nc.gpsimd.iota(gti[:], [[1, U]], base=0, channel_multiplier=U)
    nc.vector.tensor_copy(gt[:], gti[:])

    chunks = []
    o = 0
    for c in CHUNK_SIZES:
        chunks.append((o, c))
        o += c
    assert o == BL
    xts = []
    for ci, (bs, bc) in enumerate(chunks):
        xt = xp.tile([P, bc, U, N], F32, tag="xt")
        src = x[bs : bs + bc].rearrange("b (p u) n -> p b u n", p=P)
        if ci == 0:
            QU = U // 4
            for q in range(4):
                nc.sync.dma_start(
                    xt[:, :, q * QU : (q + 1) * QU, :],
                    src[:, :, q * QU : (q + 1) * QU, :],
                )
        else:
            HU = U // 2
            nc.sync.dma_start(xt[:, :, 0:HU, :], src[:, :, 0:HU, :])
            nc.sync.dma_start(xt[:, :, HU:U, :], src[:, :, HU:U, :])
        if ci == 0:
            nc.sync.dma_start(ut_t[:], utc[:])
            nc.sync.dma_start(id_t[:], idc[:])
        xts.append(xt)

    sv = scalar_prep(nc, pers, ptile)

    def sb(i, np_=P, p0=0):
        return sv[p0 : p0 + np_, i : i + 1]

    nf = pers.tile([P, U], F32)
    nc.vector.tensor_scalar_add(nf[:], gt[:], 1.0)
    zL = pers.tile([P, U], F32)
    nc.vector.tensor_scalar(zL[:], nf[:], sb(1), sb(2), ALU.mult, ALU.add)
    pvnL = pers.tile([P, U], F32)
    nc.vector.reciprocal(pvnL[:], zL[:])
    zR = pers.tile([P, U], F32)
    nc.vector.tensor_scalar(zR[:], gt[:], sb(3), sb(4), ALU.mult, ALU.add)
    pvnR = pers.tile([P, U], F32)
    nc.vector.reciprocal(pvnR[:], zR[:])
    lpvnL = pers.tile([P, U], F32)
    nc.scalar.activation(lpvnL[:], pvnL[:], AF.Ln)
    lpvnR = pers.tile([P, U], F32)
    nc.scalar.activation(lpvnR[:], pvnR[:], AF.Ln)
    kc2 = pers.tile([P, U], F32)
    nc.vector.tensor_add(kc2[:], lpvnL[:], lpvnR[:])

    nRf = pers.tile([P, U], F32)
    nc.vector.tensor_scalar(nRf[:], gt[:], -1.0, 8191.0, ALU.mult, ALU.add)
    gc = pers.tile([P, U], F32)
    nc.vector.tensor_scalar_max(gc[:], gt[:], 1.0)
    inv_n1 = pers.tile([P, U], F32)
    nc.vector.reciprocal(inv_n1[:], gc[:])
    nR1c = pers.tile([P, U], F32)
    nc.vector.tensor_scalar(nR1c[:], gt[:], -1.0, 8190.0, ALU.mult, ALU.add)
    nc.vector.tensor_scalar_max(nR1c[:], nR1c[:], 1.0)
    inv_nR1 = pers.tile([P, U], F32)
    nc.vector.reciprocal(inv_nR1[:], nR1c[:])
    inv_n = pers.tile([P, U], F32)
    nc.vector.reciprocal(inv_n[:], nf[:])
    inv_nR = pers.tile([P, U], F32)
    nRc = pers.tile([P, U], F32)
    nc.vector.tensor_scalar_max(nRc[:], nRf[:], 1.0)
    nc.vector.reciprocal(inv_nR[:], nRc[:])

    n_n1 = pers.tile([P, U], F32)
    nc.vector.tensor_mul(n_n1[:], nf[:], inv_n1[:])
    nR_nR1 = pers.tile([P, U], F32)
    nc.vector.tensor_mul(nR_nR1[:], nRf[:], inv_nR1[:])
    i_nn1 = pers.tile([P, U], F32)
    nc.vector.tensor_mul(i_nn1[:], inv_n[:], inv_n1[:])
    i_nRnR1 = pers.tile([P, U], F32)
    nc.vector.tensor_mul(i_nRnR1[:], inv_nR[:], inv_nR1[:])

    CBL = pers.tile([P, U], F32)
    nc.scalar.activation(CBL[:], n_n1[:], AF.Copy, scale=sb(7))
    CBR = pers.tile([P, U], F32)
    nc.scalar.activation(CBR[:], nR_nR1[:], AF.Copy, scale=sb(7))
    CA2L = pers.tile([P, U], F32)
    q1 = pers.tile([P, U], F32)
    nc.scalar.activation(q1[:], pvnL[:], AF.Copy, scale=sb(8))
    q2 = pers.tile([P, U], F32)
    nc.scalar.activation(q2[:], i_nn1[:], AF.Copy, scale=sb(9))
    nc.vector.tensor_add(CA2L[:], q1[:], q2[:])
    CA2R = pers.tile([P, U], F32)
    q1b = pers.tile([P, U], F32)
    nc.scalar.activation(q1b[:], pvnR[:], AF.Copy, scale=sb(8))
    q2b = pers.tile([P, U], F32)
    nc.scalar.activation(q2b[:], i_nRnR1[:], AF.Copy, scale=sb(9))
    nc.vector.tensor_add(CA2R[:], q1b[:], q2b[:])
    CAL = pers.tile([P, U], F32)
    nc.scalar.activation(CAL[:], pvnL[:], AF.Copy, scale=sb(10))
    CAR = pers.tile([P, U], F32)
    nc.scalar.activation(CAR[:], pvnR[:], AF.Copy, scale=sb(10))
    Cc = pers.tile([P, U], F32)
    p12 = pers.tile([P, U], F32)
    nc.vector.tensor_add(p12[:], pvnL[:], pvnR[:])
    cc1 = pers.tile([P, U], F32)
    nc.scalar.activation(cc1[:], p12[:], AF.Copy, scale=sb(11))
    cct = pers.tile([P, U], F32)
    nc.vector.tensor_scalar(cct[:], kc2[:], 0.5, sb(12), ALU.mult, ALU.add)
    nc.vector.tensor_add(Cc[:], cc1[:], cct[:])
    mlo = pers.tile([P, U], F32)
    nc.vector.tensor_scalar(mlo[:], gt[:], 14.5, NEG, ALU.is_lt, ALU.mult)
    mhi = pers.tile([P, U], F32)
    nc.vector.tensor_scalar(mhi[:], gt[:], 8174.5, NEG, ALU.is_ge, ALU.mult)
    nc.vector.tensor_add(Cc[:], Cc[:], mlo[:])
    nc.vector.tensor_add(Cc[:], Cc[:], mhi[:])
    nemask = pers.tile([P, U], F32)
    nc.vector.tensor_scalar(nemask[:], gt[:], 6552.5, None, ALU.is_ge)

    bund = pers.tile([P, 80], F32)
    zeros = pers.tile([P, max(CHUNK_SIZES) * U], F32)
    nc.gpsimd.memset(zeros[:], 0.0)

    Tall = ps1.tile([BL, 5, P], F32)
    Tm = Tall[:, 0, :]
    Tz = Tall[:, 1, :]
    Te = Tall[:, 2, :]
    Ta = Tall[:, 3, :]
    Tb = Tall[:, 4, :]
    M16 = pers.tile([BL, 1], F32)
    d = pers.tile([BL, P], F32)
    w = pers.tile([BL, P], F32)
    bfW = pers.tile([BL, 1], F32)
    sig = pers.tile([BL, 1], F32)

    def trace_bfw():
        nc.tensor.transpose(Ta, bund[:, 48 : 48 + BL], id_t[:])
        nc.tensor.transpose(Tb, bund[:, 64 : 64 + BL], id_t[:])
        At16 = Ta[:, 0:1]
        Bt16 = Tb[:, 0:1]
        t1 = pers.tile([BL, 1], F32)
        nc.scalar.activation(t1[:], At16, AF.Square, scale=1.0 / 32.0)
        v2 = pers.tile([BL, 1], F32)
        nc.vector.tensor_scalar_mul(v2[:], t1[:], 1.0 / 8192.0)
        vW = pers.tile([BL, 1], F32)
        nc.vector.scalar_tensor_tensor(
            vW[:], Bt16, 1.0 / 1024.0, v2[:], ALU.mult, ALU.subtract
        )
        nc.vector.tensor_scalar(vW[:], vW[:], 1.0 / 8191.0, 1.0e-8, ALU.mult, ALU.max)
        term1 = pers.tile([BL, 1], F32)
        nc.vector.tensor_scalar_mul(term1[:], vW[:], sb(17, BL))
        term2 = pers.tile([BL, 1], F32)
        nc.vector.tensor_scalar_mul(term2[:], t1[:], sb(18, BL))
        uu = pers.tile([BL, 1], F32)
        nc.scalar.activation(uu[:], At16, AF.Identity, bias=sb(6, BL), scale=sb(5, BL))
        u2 = pers.tile([BL, 1], F32)
        nc.scalar.activation(u2[:], uu[:], AF.Square)
        term3 = pers.tile([BL, 1], F32)
        nc.vector.tensor_scalar_mul(term3[:], u2[:], sb(13, BL))
        tsum = pers.tile([BL, 1], F32)
        nc.vector.tensor_add(tsum[:], term1[:], term2[:])
        nc.vector.tensor_sub(tsum[:], tsum[:], term3[:])
        nc.vector.tensor_scalar(bfW[:], tsum[:], -0.5, sb(19, BL), ALU.mult, ALU.add)

    def trace_maxw():
        nc.tensor.transpose(Tm, bund[:, 0:BL], id_t[:])
        nc.vector.tensor_reduce(M16[:], Tm, AX.X, ALU.min, negate=True)
        nc.vector.tensor_scalar(d[:], Tm, -1.0, M16[:], ALU.mult, ALU.subtract)
        nc.scalar.activation(w[:], d[:], AF.Exp)
        sigin = pers.tile([BL, 1], F32)
        nc.vector.tensor_sub(sigin[:], M16[:], bfW[:])
        nc.scalar.activation(sig[:], sigin[:], AF.Sigmoid)

    xhs = [None] * len(chunks)

    def ufof(ci):
        return UF_SCHED[ci] if UF_SCHED is not None else 32

    def trace_fold(ci):
        uf = ufof(ci)
        bc = chunks[ci][1]
        if uf > 0:
            xh = wk.tile([P, bc, uf, 16], F32, tag="xh")
            nc.gpsimd.tensor_add(
                xh[:], xts[ci][:, :, 0:uf, 0:16], xts[ci][:, :, 0:uf, 16:32]
            )
            xhs[ci] = xh

    trace_fold(0)
    for ci, (bs, bc) in enumerate(chunks):
        xt = xts[ci]
        last = ci == len(chunks) - 1
        if not last:
            trace_fold(ci + 1)

        uf = ufof(ci)
        sr = wk.tile([P, bc, U], F32)
        if uf > 0:
            if ci == 0 and uf == 16:
                for q in range(1, 4):
                    nc.vector.tensor_reduce(
                        sr[:, :, q * 16 : (q + 1) * 16],
                        xt[:, :, q * 16 : (q + 1) * 16, :],
                        AX.X,
                        ALU.add,
                    )
            elif uf < U:
                nc.vector.tensor_reduce(
                    sr[:, :, uf:U], xt[:, :, uf:U, :], AX.X, ALU.add
                )
            nc.vector.tensor_reduce(sr[:, :, 0:uf], xhs[ci][:], AX.X, ALU.add)
        else:
            HU = U // 2
            nc.vector.tensor_reduce(
                sr[:, :, 0:HU], xt[:, :, 0:HU, :], AX.X, ALU.add
            )
            nc.vector.tensor_reduce(
                sr[:, :, HU:U], xt[:, :, HU:U, :], AX.X, ALU.add
            )
        sq = wk.tile([P, bc, U], F32)
        nc.scalar.activation(sq[:], sr[:], AF.Square)

        A = wk.tile([P, bc, U], F32)
        nc.vector.tensor_tensor_scan(
            A[:].rearrange("p b u -> p (b u)"),
            sr[:].rearrange("p b u -> p (b u)"),
            zeros[:, 0 : bc * U],
            0.0,
            ALU.add,
            ALU.add,
        )
        Bt_ = wk.tile([P, bc, U], F32)
        nc.vector.tensor_tensor_scan(
            Bt_[:].rearrange("p b u -> p (b u)"),
            sq[:].rearrange("p b u -> p (b u)"),
            zeros[:, 0 : bc * U],
            0.0,
            ALU.add,
            ALU.add,
        )

        rv = wk.tile([P, 2 * bc], F32)
        nc.gpsimd.memset(rv[:, 0:1], 0.0)
        nc.gpsimd.memset(rv[:, bc : bc + 1], 0.0)
        nc.vector.tensor_copy(rv[:, 1:bc], A[:, 0 : bc - 1, U - 1])
        nc.vector.tensor_copy(rv[:, bc + 1 : 2 * bc], Bt_[:, 0 : bc - 1, U - 1])
        ct = wk.tile([P, 2 * bc], F32)
        nc.vector.tensor_sub(ct[:, 0:bc], A[:, :, U - 1], rv[:, 0:bc])
        nc.vector.tensor_sub(ct[:, bc : 2 * bc], Bt_[:, :, U - 1], rv[:, bc : 2 * bc])
        g_ps = psp.tile([P, 2 * bc], F32)
        nc.tensor.matmul(g_ps[:], ut_t[:], ct[:])
        tot_ps = psp.tile([P, 2 * bc], F32)
        nc.tensor.matmul(tot_ps[:], ones_t[:], ct[:])
        off = wk.tile([P, 2 * bc], F32)
        nc.vector.tensor_sub(off[:], g_ps[:], rv[:])

        offA_b = off[:, 0:bc].unsqueeze(2).broadcast_to([P, bc, U])
        offB_b = off[:, bc : 2 * bc].unsqueeze(2).broadcast_to([P, bc, U])
        nc.vector.tensor_add(A[:], A[:], offA_b)
        nc.vector.tensor_add(Bt_[:], Bt_[:], offB_b)

        nc.scalar.copy(bund[:, 48 + bs : 48 + bs + bc], tot_ps[:, 0:bc])
        nc.scalar.copy(bund[:, 64 + bs : 64 + bs + bc], tot_ps[:, bc : 2 * bc])
        if last:
            trace_bfw()
        At_b = (
            bund[:, 48 + bs : 48 + bs + bc].unsqueeze(2).broadcast_to([P, bc, U])
        )
        Btot_b = (
            bund[:, 64 + bs : 64 + bs + bc].unsqueeze(2).broadcast_to([P, bc, U])
        )
        AR = wk.tile([P, bc, U], F32)
        nc.gpsimd.tensor_sub(AR[:], At_b, A[:])
        BR = wk.tile([P, bc, U], F32)
        nc.gpsimd.tensor_sub(BR[:], Btot_b, Bt_[:])

        A2 = wk.tile([P, bc, U], F32)
        nc.scalar.activation(A2[:], A[:], AF.Square)
        AR2 = wk.tile([P, bc, U], F32)
        nc.scalar.activation(AR2[:], AR[:], AF.Square)

        def cb(t):
            return t[:].unsqueeze(1).broadcast_to([P, bc, U])

        bf = wk.tile([P, bc, U], F32)
        p2 = wk.tile([P, bc, U], F32)
        p3 = wk.tile([P, bc, U], F32)
        p5 = wk.tile([P, bc, U], F32)
        p6 = wk.tile([P, bc, U], F32)
        nc.vector.tensor_mul(p2[:], A2[:], cb(CA2L))
        nc.vector.tensor_mul(p3[:], Bt_[:], cb(CBL))
        nc.vector.tensor_mul(p5[:], AR2[:], cb(CA2R))
        (nc.vector if last else nc.gpsimd).tensor_mul(p6[:], BR[:], cb(CBR))
        p1 = wk.tile([P, bc, U], F32)
        p4 = wk.tile([P, bc, U], F32)
        nc.vector.tensor_mul(p1[:], A[:], cb(CAL))
        nc.vector.tensor_mul(p4[:], AR[:], cb(CAR))
        nc.gpsimd.tensor_add(p1[:], p1[:], p2[:])
        nc.vector.tensor_add(p3[:], p3[:], p4[:])
        nc.gpsimd.tensor_add(p5[:], p5[:], p6[:])
        nc.gpsimd.tensor_add(p1[:], p1[:], cb(Cc))
        nc.vector.tensor_add(p3[:], p3[:], p5[:])
        nc.vector.tensor_add(bf[:], p1[:], p3[:])

        nc.vector.tensor_reduce(
            bund[:, bs : bs + bc], bf[:], AX.X, ALU.max, negate=True
        )
        if last:
            trace_maxw()
        e = wk.tile([P, bc, U], F32)
        for b in range(bc):
            nc.scalar.activation(
                e[:, b, :],
                bf[:, b, :],
                AF.Exp,
                bias=bund[:, bs + b : bs + b + 1],
                accum_out=bund[:, 16 + bs + b : 17 + bs + b],
            )
        en = wk.tile([P, bc, U], F32)
        (nc.vector if last else nc.gpsimd).tensor_mul(en[:], e[:], cb(nemask))
        nc.vector.tensor_reduce(
            bund[:, 32 + bs : 32 + bs + bc], en[:], AX.X, ALU.add
        )

    nc.tensor.transpose(Tz, bund[:, 16 : 16 + BL], id_t[:])
    nc.tensor.transpose(Te, bund[:, 32 : 32 + BL], id_t[:])
    wz = pers.tile([BL, P], F32)
    Zb = pers.tile([BL, 1], F32)
    nc.vector.scalar_tensor_tensor(
        wz[:], w[:], 1.0, Tz, ALU.mult, ALU.mult, accum_out=Zb[:]
    )
    wn = pers.tile([BL, P], F32)
    Nb = pers.tile([BL, 1], F32)
    nc.vector.scalar_tensor_tensor(
        wn[:], w[:], 1.0, Te, ALU.mult, ALU.mult, accum_out=Nb[:]
    )
    invZ = pers.tile([BL, 1], F32)
    nc.vector.reciprocal(invZ[:], Zb[:])
    ratio = pers.tile([BL, 1], F32)
    nc.vector.tensor_mul(ratio[:], Nb[:], invZ[:])
    outv = pers.tile([BL, 1], F32)
    nc.vector.tensor_mul(outv[:], sig[:], ratio[:])
    nc.sync.dma_start(out[:], outv[:])


def host_consts():
    ut = np.triu(np.ones((P, P), np.float32), 1)
    ident = np.eye(P, dtype=np.float32)
    return ut, ident


def split_multi_waits(nc):
    """Walrus in this container allows one sync wait per instruction; move
    extra waits onto same-engine NOPs inserted immediately before."""
    import bass_rust

    nid = [0]
    for f in nc.m.functions:
        for b in f.blocks:
            insts = b.instructions
            i = 0
            while i < len(insts):
                ins = insts[i]
                si = ins.sync_info
                if si is not None and si.on_wait is not None and len(si.on_wait) > 1:
                    waits = list(si.on_wait)
                    for w in waits[:-1]:
                        nop = mybir.InstNoOp(
                            name=f"I-waitsplit-{nid[0]}", ins=[], outs=[]
                        )
                        nid[0] += 1
                        nop.engine = ins.engine
                        nop.sync_info = bass_rust.SyncInfo(
                            on_wait=[w], on_update=[]
                        )
                        insts.insert(i, nop)
                        i += 1
                    si.on_wait = waits[-1:]
                i += 1


_NC_CACHE = {}


def build_nc(split=True, reps=1, pm_zero=False):
    global _NC_CACHE
    key = (split, reps, pm_zero)
    if key in _NC_CACHE:
        return _NC_CACHE[key]
    nc = bass.Bass()
    x = nc.declare_dram_parameter("x", [BL, T, N], F32, isOutput=False)
    params = nc.declare_dram_parameter("params", [P, 3], F32, isOutput=False)
    utc = nc.declare_dram_parameter("utc", [P, P], F32, isOutput=False)
    idc = nc.declare_dram_parameter("idc", [P, P], F32, isOutput=False)
    out = nc.declare_dram_parameter("out", [BL, 1], F32, isOutput=True)
    with tile.TileContext(nc) as tc:
        for _ in range(reps):
            with ExitStack() as ctx:
                if pm_zero:
                    build_body_pm0(
                        ctx, tc, x[:], params[:], utc[:], idc[:], out[:]
                    )
                else:
                    build_body(
                        ctx, tc, x[:], params[:], utc[:], idc[:], out[:]
                    )
    if split:
        split_multi_waits(nc)
    _NC_CACHE[key] = nc
    return nc


def make_in_maps(x, prior_mean, prior_var, noise_var):
    x = np.ascontiguousarray(np.asarray(x, dtype=np.float32))
    params = np.tile(
        np.array(
            [[float(prior_mean[0]), float(prior_var[0]), float(noise_var[0])]],
            dtype=np.float32,
        ),
        (P, 1),
    )
    ut, ident = host_consts()
    in_maps = []
    for c in range(NCORES):
        in_maps.append(
            {
                "x": x[c * BL : (c + 1) * BL],
                "params": params,
                "utc": ut,
                "idc": ident,
            }
        )
    return in_maps


def kernel(x, prior_mean, prior_var, noise_var):
    from concourse.bass_utils import run_bass_kernel_spmd

    in_maps = make_in_maps(x, prior_mean, prior_var, noise_var)
    nc = build_nc(pm_zero=(float(np.asarray(prior_mean).reshape(-1)[0]) == 0.0))
    res = run_bass_kernel_spmd(nc, in_maps, list(range(NCORES)))
    outs = [np.asarray(res.results[c]["out"]).reshape(BL) for c in range(NCORES)]
    return np.concatenate(outs).astype(np.float32)


# revision 13
# speedup vs baseline: 1.2436x; 1.0023x over previous
"""Trainium2 Bass kernel for BayesianChangePointDetector (segment_reduce).

Contract: kernel(**inputs) takes FULL inputs (x:[128,8192,32] f32, plus 3
scalar prior params) and returns the FULL [128] f32 output. Internally the
batch dim is sharded across 8 NeuronCores (16 rows each, pure data parallel,
no collectives), each core runs the same Bass/Tile program, and the host
concatenates the 8 per-core [16] outputs.

Fast path (pm == 0, the shipped input): x is cast f32->fp16 during the SWDGE
DMA (halves the charged DMA time), the N=32 fold is a fp16 binary add-tree
(32->16->8->4 on DVE in 2x perf mode, 4->2->1 on Pool), prefix sums are
per-row f32 scans with a triangular-ones carry matmul on PE, and the Bayes
factor is assembled as bf'' = CA2L*A^2 + CA2R*(At-A)^2 + SB*B + CBR2*Btot[b]
+ Cc_var with all large per-row constants moved into the final sigmoid path,
so bf'' is O(10) and exp needs no max-shift. A^2 and (At-A)^2 come from ACT
Square activations whose bias folds in the carry offsets.
"""

import sys

if "/opt/trn_rl_repo" not in sys.path:
    sys.path.insert(0, "/opt/trn_rl_repo")

import math
from contextlib import ExitStack

import numpy as np

import concourse.bass as bass
import concourse.tile as tile
from concourse import mybir

F32 = mybir.dt.float32
F16 = mybir.dt.float16
AF = mybir.ActivationFunctionType
ALU = mybir.AluOpType
AX = mybir.AxisListType

B, T, N = 128, 8192, 32
NCORES = 8
BL = B // NCORES  # 16 batch rows per core
P = 128           # partitions = t-blocks
U = T // P        # 64 t's per partition
NS = 32           # scalar-slot count
NEG = -1.0e30

# fast-path chunking: small last chunk to shrink the post-DMA tail
CHUNKS_PM0 = [(0, 4), (4, 5), (9, 5), (14, 2)]

# near-end threshold: candidates g >= 6553 (g = 64p+u); p=102 partial (u>=25),
# p>=103 fully near-end.
NE_P = 102
NE_U = 25
# valid candidates: g in [15, 8175)
LO_THR = 14.5
HI_THR = 8174.5


def scalar_prep(nc, pers, ptile):
    """Per-partition scalar slots, identical math to the baseline kernel."""
    sv = pers.tile([P, NS], F32)
    tmp = pers.tile([P, 8], F32)

    def s(i):
        return sv[:, i : i + 1]

    def tm(i):
        return tmp[:, i : i + 1]

    # softplus(x) = ln(1 + exp(x))
    nc.scalar.activation(tm(0), ptile[:, 1:2], AF.Exp)
    nc.vector.tensor_scalar_add(tm(0), tm(0), 1.0)
    nc.scalar.activation(s(20), tm(0), AF.Ln)
    nc.scalar.activation(tm(1), ptile[:, 2:3], AF.Exp)
    nc.vector.tensor_scalar_add(tm(1), tm(1), 1.0)
    nc.scalar.activation(s(21), tm(1), AF.Ln)
    nc.vector.tensor_copy(s(0), ptile[:, 0:1])
    nc.vector.reciprocal(s(1), s(21))
    nc.vector.reciprocal(s(2), s(20))
    nc.vector.tensor_scalar_mul(s(3), s(1), -1.0)
    nc.vector.tensor_scalar(s(4), s(1), 8191.0, s(2), ALU.mult, ALU.add)
    nc.vector.tensor_scalar_mul(s(5), s(1), 1.0 / 32.0)
    nc.vector.tensor_mul(s(6), s(0), s(2))
    nc.vector.tensor_scalar_mul(s(7), s(1), -0.5 / 1024.0)
    nc.vector.tensor_scalar_mul(s(9), s(1), 0.5 / 1024.0)
    nc.vector.tensor_mul(tm(0), s(5), s(5))
    nc.vector.tensor_scalar_mul(s(8), tm(0), 0.5)
    nc.vector.tensor_mul(s(10), s(6), s(5))
    nc.vector.tensor_mul(tm(1), s(6), s(6))
    nc.vector.tensor_scalar_mul(s(11), tm(1), 0.5)
    nc.scalar.activation(s(14), s(21), AF.Ln, scale=2.0 * math.pi)
    nc.scalar.activation(s(15), s(20), AF.Ln)
    nc.vector.tensor_scalar_mul(s(17), s(1), 8192.0)
    nc.vector.tensor_scalar(tm(2), s(1), 8192.0, s(2), ALU.mult, ALU.add)
    nc.vector.reciprocal(s(13), tm(2))
    nc.scalar.activation(s(16), s(13), AF.Ln)
    nc.vector.tensor_scalar_mul(s(18), s(1), 1.0 / 8192.0)
    nc.vector.tensor_mul(tm(3), s(0), s(0))
    nc.vector.tensor_mul(s(22), tm(3), s(2))
    nc.vector.tensor_scalar_mul(s(23), s(14), -4096.0)
    nc.vector.tensor_sub(tm(4), s(23), s(15))
    nc.vector.tensor_sub(s(12), tm(4), s(22))
    nc.vector.tensor_sub(tm(5), s(16), s(15))
    nc.vector.tensor_scalar_mul(tm(5), tm(5), 0.5)
    nc.vector.tensor_add(tm(6), s(23), tm(5))
    nc.vector.tensor_scalar_mul(tm(7), s(22), -0.5)
    nc.vector.tensor_add(s(19), tm(6), tm(7))
    return sv


def build_body_pm0(ctx, tc, x, params, utc, idc, out):
    nc = tc.nc
    pers = ctx.enter_context(tc.tile_pool(name="pers", bufs=1))
    xp = ctx.enter_context(tc.tile_pool(name="xp", bufs=2))
    wk = ctx.enter_context(tc.tile_pool(name="wk", bufs=3))
    psp = ctx.enter_context(tc.tile_pool(name="psp", bufs=2, space="PSUM"))
    ps1 = ctx.enter_context(tc.tile_pool(name="ps1", bufs=1, space="PSUM"))

    chunks = CHUNKS_PM0
    nch = len(chunks)

    # ---------- consts + early DMAs ----------
    ut_t = pers.tile([P, P], F32)
    ones_t = pers.tile([P, P], F32)
    id_t = pers.tile([P, P], F32)
    gt = pers.tile([P, U], F32)
    ptile = pers.tile([P, 3], F32)
    bund = pers.tile([P, 80], F32)  # [0:16) -rmax | [16:32) Zp | [32:48) En | [48:64) At | [64:80) Btot
    zeros = pers.tile([P, U], F32)

    nc.sync.dma_start(ptile[:], params[:])

    xts = [None] * nch

    def issue_dma(ci):
        bs, bc = chunks[ci]
        xt = xp.tile([P, bc, U, N], F16, tag="xt")
        src = x[bs : bs + bc].rearrange("b (p u) n -> p b u n", p=P)
        if ci == 0:
            HU = U // 2
            nc.gpsimd.dma_start(xt[:, :, 0:HU, :], src[:, :, 0:HU, :])
            nc.gpsimd.dma_start(xt[:, :, HU:U, :], src[:, :, HU:U, :])
        else:
            nc.gpsimd.dma_start(xt[:], src[:])
        xts[ci] = xt

    # Pool stream: iota, chunk0 + chunk1 DMA descriptors, then memsets
    gti = pers.tile([P, U], mybir.dt.int32)
    nc.gpsimd.iota(gti[:], [[1, U]], base=0, channel_multiplier=U)
    issue_dma(0)
    issue_dma(1)
    nc.gpsimd.memset(ones_t[:], 1.0)
    nc.gpsimd.memset(zeros[:], 0.0)

    # HWDGE consts behind params
    nc.sync.dma_start(ut_t[:], utc[:])
    nc.sync.dma_start(id_t[:], idc[:])

    nc.vector.tensor_copy(gt[:], gti[:])
    sv = scalar_prep(nc, pers, ptile)

    def sb(i, np_=P, p0=0):
        return sv[p0 : p0 + np_, i : i + 1]

    # ---------- per-candidate coefficient vectors [P, U] ----------
    nf = pers.tile([P, U], F32)
    nc.vector.tensor_scalar_add(nf[:], gt[:], 1.0)
    zL = pers.tile([P, U], F32)
    nc.vector.tensor_scalar(zL[:], nf[:], sb(1), sb(2), ALU.mult, ALU.add)
    pvnL = pers.tile([P, U], F32)
    nc.vector.reciprocal(pvnL[:], zL[:])
    zR = pers.tile([P, U], F32)
    nc.vector.tensor_scalar(zR[:], gt[:], sb(3), sb(4), ALU.mult, ALU.add)
    pvnR = pers.tile([P, U], F32)
    nc.vector.reciprocal(pvnR[:], zR[:])
    lpvnL = pers.tile([P, U], F32)
    nc.scalar.activation(lpvnL[:], pvnL[:], AF.Ln)
    lpvnR = pers.tile([P, U], F32)
    nc.scalar.activation(lpvnR[:], pvnR[:], AF.Ln)
    # Cc_var = 0.5*(ln pvnL + ln pvnR) + validity masks (NO sc constant here)
    Ccv = pers.tile([P, U], F32)
    nc.vector.tensor_add(Ccv[:], lpvnL[:], lpvnR[:])
    nc.vector.tensor_scalar_mul(Ccv[:], Ccv[:], 0.5)
    mlo = pers.tile([P, U], F32)
    nc.vector.tensor_scalar(mlo[:], gt[:], LO_THR, NEG, ALU.is_lt, ALU.mult)
    mhi = pers.tile([P, U], F32)
    nc.vector.tensor_scalar(mhi[:], gt[:], HI_THR, NEG, ALU.is_ge, ALU.mult)
    nc.vector.tensor_add(Ccv[:], Ccv[:], mlo[:])
    nc.vector.tensor_add(Ccv[:], Ccv[:], mhi[:])

    gc = pers.tile([P, U], F32)
    nc.vector.tensor_scalar_max(gc[:], gt[:], 1.0)
    inv_n1 = pers.tile([P, U], F32)
    nc.vector.reciprocal(inv_n1[:], gc[:])
    nR1c = pers.tile([P, U], F32)
    nc.vector.tensor_scalar(nR1c[:], gt[:], -1.0, 8190.0, ALU.mult, ALU.add)
    nc.vector.tensor_scalar_max(nR1c[:], nR1c[:], 1.0)
    inv_nR1 = pers.tile([P, U], F32)
    nc.vector.reciprocal(inv_nR1[:], nR1c[:])
    inv_n = pers.tile([P, U], F32)
    nc.vector.reciprocal(inv_n[:], nf[:])
    nRf = pers.tile([P, U], F32)
    nc.vector.tensor_scalar(nRf[:], gt[:], -1.0, 8191.0, ALU.mult, ALU.add)
    nRc = pers.tile([P, U], F32)
    nc.vector.tensor_scalar_max(nRc[:], nRf[:], 1.0)
    inv_nR = pers.tile([P, U], F32)
    nc.vector.reciprocal(inv_nR[:], nRc[:])

    n_n1 = pers.tile([P, U], F32)
    nc.vector.tensor_mul(n_n1[:], nf[:], inv_n1[:])
    nR_nR1 = pers.tile([P, U], F32)
    nc.vector.tensor_mul(nR_nR1[:], nRf[:], inv_nR1[:])
    i_nn1 = pers.tile([P, U], F32)
    nc.vector.tensor_mul(i_nn1[:], inv_n[:], inv_n1[:])
    i_nRnR1 = pers.tile([P, U], F32)
    nc.vector.tensor_mul(i_nRnR1[:], inv_nR[:], inv_nR1[:])

    # CA2L = (k^2/2)*pvnL + (kq/2)*i_nn1 ; CA2R analog
    CA2L = pers.tile([P, U], F32)
    q1 = pers.tile([P, U], F32)
    nc.scalar.activation(q1[:], pvnL[:], AF.Copy, scale=sb(8))
    q2 = pers.tile([P, U], F32)
    nc.scalar.activation(q2[:], i_nn1[:], AF.Copy, scale=sb(9))
    nc.vector.tensor_add(CA2L[:], q1[:], q2[:])
    CA2R = pers.tile([P, U], F32)
    q1b = pers.tile([P, U], F32)
    nc.scalar.activation(q1b[:], pvnR[:], AF.Copy, scale=sb(8))
    q2b = pers.tile([P, U], F32)
    nc.scalar.activation(q2b[:], i_nRnR1[:], AF.Copy, scale=sb(9))
    nc.vector.tensor_add(CA2R[:], q1b[:], q2b[:])
    # SB = CBL - CBR = (-kq/2)*(n/(n-1) - nR/(nR-1))
    dsb = pers.tile([P, U], F32)
    nc.vector.tensor_sub(dsb[:], n_n1[:], nR_nR1[:])
    SBt = pers.tile([P, U], F32)
    nc.scalar.activation(SBt[:], dsb[:], AF.Copy, scale=sb(7))
    # CBR2 = (-kq/2) * 1/(nR-1)
    CBR2 = pers.tile([P, U], F32)
    nc.scalar.activation(CBR2[:], inv_nR1[:], AF.Copy, scale=sb(7))
    # fp16 copies of the A^2 coefficients, pre-scaled by 64^2 to match the
    # 1/64-scaled squares (products are exact-scale); ranges stay normal fp16
    CA2Lh = pers.tile([P, U], F16)
    nc.scalar.activation(CA2Lh[:], CA2L[:], AF.Copy, scale=4096.0)
    CA2Rh = pers.tile([P, U], F16)
    nc.scalar.activation(CA2Rh[:], CA2R[:], AF.Copy, scale=4096.0)

    # ---------- finale tiles ----------
    Tall = ps1.tile([BL, 5, P], F32)
    Tm = Tall[:, 0, :]
    Tz = Tall[:, 1, :]
    Te = Tall[:, 2, :]
    Ta = Tall[:, 3, :]
    Tb = Tall[:, 4, :]
    M16 = pers.tile([BL, 1], F32)
    bfW = pers.tile([BL, 1], F32)
    sig = pers.tile([BL, 1], F32)

    def cbc(t, bc):
        return t[:].unsqueeze(1).broadcast_to([P, bc, U])

    # ---------- per-chunk pipeline ----------
    st = [dict() for _ in range(nch)]

    def emit_folds(ci):
        bs, bc = chunks[ci]
        xt = xts[ci]
        h1 = wk.tile([P, bc, U, 16], F16, tag="h1")
        if ci == 0:
            HU = U // 2
            nc.vector.tensor_add(
                h1[:, :, 0:HU, :], xt[:, :, 0:HU, 0:16], xt[:, :, 0:HU, 16:32]
            )
            nc.vector.tensor_add(
                h1[:, :, HU:U, :], xt[:, :, HU:U, 0:16], xt[:, :, HU:U, 16:32]
            )
        else:
            nc.vector.tensor_add(h1[:], xt[:, :, :, 0:16], xt[:, :, :, 16:32])
        h2 = wk.tile([P, bc, U, 8], F16, tag="h2")
        nc.vector.tensor_add(h2[:], h1[:, :, :, 0:8], h1[:, :, :, 8:16])
        h3 = wk.tile([P, bc, U, 4], F16, tag="h3")
        nc.vector.tensor_add(h3[:], h2[:, :, :, 0:4], h2[:, :, :, 4:8])
        h4 = wk.tile([P, bc, U, 2], F16, tag="h4")
        nc.gpsimd.tensor_add(h4[:], h3[:, :, :, 0:2], h3[:, :, :, 2:4])
        sr = wk.tile([P, bc, U], F32, tag="sr")
        nc.gpsimd.tensor_add(sr[:], h4[:, :, :, 0:1], h4[:, :, :, 1:2])
        st[ci]["sr"] = sr

    def emit_sq(ci):
        bs, bc = chunks[ci]
        sq = wk.tile([P, bc, U], F32, tag="sq")
        nc.scalar.activation(sq[:], st[ci]["sr"][:], AF.Square)
        st[ci]["sq"] = sq

    emit_folds(0)
    for ci, (bs, bc) in enumerate(chunks):
        last = ci == nch - 1
        sr = st[ci]["sr"]
        if "sq" not in st[ci]:
            emit_sq(ci)
        sq = st[ci]["sq"]

        A = wk.tile([P, bc, U], F32, tag="A")
        Bt_ = wk.tile([P, bc, U], F32, tag="B")
        for b in range(bc):
            nc.vector.tensor_tensor_scan(
                A[:, b, :], sr[:, b, :], zeros[:], 0.0, ALU.add, ALU.add
            )
        for b in range(bc):
            nc.vector.tensor_tensor_scan(
                Bt_[:, b, :], sq[:, b, :], zeros[:], 0.0, ALU.add, ALU.add
            )

        # carry fix: per-row totals -> triangular matmul
        ct = wk.tile([P, 2 * bc], F32, tag="ct")
        nc.scalar.copy(ct[:, 0:bc], A[:, :, U - 1])
        nc.scalar.copy(ct[:, bc : 2 * bc], Bt_[:, :, U - 1])
        g_ps = psp.tile([P, 2 * bc], F32, tag="gps")
        nc.tensor.matmul(g_ps[:], ut_t[:], ct[:])
        tot_ps = psp.tile([P, 2 * bc], F32, tag="tot")
        nc.tensor.matmul(tot_ps[:], ones_t[:], ct[:])

        offs = wk.tile([P, 2 * bc], F32, tag="offs")
        nc.scalar.copy(offs[:], g_ps[:])
        nc.scalar.copy(bund[:, 48 + bs : 48 + bs + bc], tot_ps[:, 0:bc])
        nc.scalar.copy(bund[:, 64 + bs : 64 + bs + bc], tot_ps[:, bc : 2 * bc])

        # carry offsets applied as broadcast adds (Pool), then Pool m3
        offA_b = offs[:, 0:bc].unsqueeze(2).broadcast_to([P, bc, U])
        nc.gpsimd.tensor_add(A[:], A[:], offA_b)
        offB_b = offs[:, bc : 2 * bc].unsqueeze(2).broadcast_to([P, bc, U])
        nc.gpsimd.tensor_add(Bt_[:], Bt_[:], offB_b)
        nc.gpsimd.tensor_mul(Bt_[:], Bt_[:], cbc(SBt, bc))  # m3 in-place

        # cross-chunk interleave: next chunk's folds + next-next DMA issue
        # ahead of this chunk's tail so engine queues never drain
        if ci + 1 < nch:
            emit_folds(ci + 1)
        if ci + 2 < nch:
            issue_dma(ci + 2)

        # AR = At - A_true (DVE), then single-op fp16 squares on ACT
        ARt = wk.tile([P, bc, U], F32, tag="ARt")
        At_b = bund[:, 48 + bs : 48 + bs + bc].unsqueeze(2).broadcast_to([P, bc, U])
        nc.vector.tensor_sub(ARt[:], At_b, A[:])
        A2 = wk.tile([P, bc, U], F16, tag="A2")
        nc.scalar.activation(A2[:], A[:], AF.Square, scale=1.0 / 64.0)
        AR2 = wk.tile([P, bc, U], F16, tag="AR2")
        nc.scalar.activation(AR2[:], ARt[:], AF.Square, scale=1.0 / 64.0)

        # bf'' = CA2L*A2 + CA2R*AR2 + m3 + (CBR2*Btot[b] + Cc_var)
        nc.vector.tensor_mul(A2[:], A2[:], cbc(CA2Lh, bc))
        nc.vector.tensor_mul(AR2[:], AR2[:], cbc(CA2Rh, bc))
        nc.vector.tensor_add(A2[:], A2[:], AR2[:])
        Kc = wk.tile([P, bc, U], F32, tag="Kc")
        for b in range(bc):
            nc.vector.scalar_tensor_tensor(
                Kc[:, b, :], CBR2[:], bund[:, 64 + bs + b : 65 + bs + b],
                Ccv[:], ALU.mult, ALU.add,
            )
        nc.vector.tensor_add(Bt_[:], Bt_[:], Kc[:])
        nc.vector.tensor_add(Bt_[:], Bt_[:], A2[:])  # Bt_ now holds bf'' (f32)

        # -max(bf'') per (p,b) for the confidence path
        nc.vector.tensor_reduce(
            bund[:, bs : bs + bc], Bt_[:], AX.X, ALU.max, negate=True
        )
        if ci + 1 < nch:
            emit_sq(ci + 1)

        # e = exp(bf'') raw, Zp accumulated per b
        e = wk.tile([P, bc, U], F32, tag="e")
        for b in range(bc):
            nc.scalar.activation(
                e[:, b, :], Bt_[:, b, :], AF.Exp,
                accum_out=bund[:, 16 + bs + b : 17 + bs + b],
            )
        # per-(p,b) tail sums over u >= NE_U; only row p=102 is used (the
        # finale takes full Zp rows for p >= 103 from the Tz transpose)
        nc.vector.tensor_reduce(
            bund[:, 32 + bs : 32 + bs + bc], e[:, :, NE_U:U], AX.X, ALU.add
        )

    # ---------- finale ----------
    # whole-window log marginal (needs At/Btot of all rows)
    nc.tensor.transpose(Ta, bund[:, 48 : 48 + BL], id_t[:])
    nc.tensor.transpose(Tb, bund[:, 64 : 64 + BL], id_t[:])
    At16 = Ta[:, 0:1]
    Bt16 = Tb[:, 0:1]
    t1 = pers.tile([BL, 1], F32)
    nc.scalar.activation(t1[:], At16, AF.Square, scale=1.0 / 32.0)
    v2 = pers.tile([BL, 1], F32)
    nc.vector.tensor_scalar_mul(v2[:], t1[:], 1.0 / 8192.0)
    vW = pers.tile([BL, 1], F32)
    nc.vector.scalar_tensor_tensor(
        vW[:], Bt16, 1.0 / 1024.0, v2[:], ALU.mult, ALU.subtract
    )
    nc.vector.tensor_scalar(vW[:], vW[:], 1.0 / 8191.0, 1.0e-8, ALU.mult, ALU.max)
    term1 = pers.tile([BL, 1], F32)
    nc.vector.tensor_scalar_mul(term1[:], vW[:], sb(17, BL))
    term2 = pers.tile([BL, 1], F32)
    nc.vector.tensor_scalar_mul(term2[:], t1[:], sb(18, BL))
    uu = pers.tile([BL, 1], F32)
    nc.scalar.activation(uu[:], At16, AF.Identity, bias=sb(6, BL), scale=sb(5, BL))
    u2 = pers.tile([BL, 1], F32)
    nc.scalar.activation(u2[:], uu[:], AF.Square)
    term3 = pers.tile([BL, 1], F32)
    nc.vector.tensor_scalar_mul(term3[:], u2[:], sb(13, BL))
    tsum = pers.tile([BL, 1], F32)
    nc.vector.tensor_add(tsum[:], term1[:], term2[:])
    nc.vector.tensor_sub(tsum[:], tsum[:], term3[:])
    nc.vector.tensor_scalar(bfW[:], tsum[:], -0.5, sb(19, BL), ALU.mult, ALU.add)
    # cW = sc - (kq/2)*Btot - bfW ; true max = M16 + cW
    cw1 = pers.tile([BL, 1], F32)
    nc.vector.scalar_tensor_tensor(
        cw1[:], Bt16, sb(9, BL), bfW[:], ALU.mult, ALU.add
    )
    cW = pers.tile([BL, 1], F32)
    nc.vector.tensor_scalar(cW[:], cw1[:], -1.0, sb(12, BL), ALU.mult, ALU.add)

    nc.tensor.transpose(Tm, bund[:, 0:BL], id_t[:])
    nc.vector.tensor_reduce(M16[:], Tm, AX.X, ALU.min, negate=True)
    sigin = pers.tile([BL, 1], F32)
    nc.vector.tensor_add(sigin[:], M16[:], cW[:])
    nc.scalar.activation(sig[:], sigin[:], AF.Sigmoid)

    nc.tensor.transpose(Tz, bund[:, 16 : 16 + BL], id_t[:])
    nc.tensor.transpose(Te, bund[:, 32 : 32 + BL], id_t[:])
    Zb = pers.tile([BL, 1], F32)
    nc.vector.tensor_reduce(Zb[:], Tz, AX.X, ALU.add)
    # near-end mass: full Zp rows p in [103,128) plus the p=102 tail sum
    Nbh = pers.tile([BL, 1], F32)
    nc.vector.tensor_reduce(Nbh[:], Tz[:, NE_P + 1 : P], AX.X, ALU.add)
    Nb = pers.tile([BL, 1], F32)
    nc.vector.tensor_add(Nb[:], Nbh[:], Te[:, NE_P : NE_P + 1])
    invZ = pers.tile([BL, 1], F32)
    nc.vector.reciprocal(invZ[:], Zb[:])
    ratio = pers.tile([BL, 1], F32)
    nc.vector.tensor_mul(ratio[:], Nb[:], invZ[:])
    outv = pers.tile([BL, 1], F32)
    nc.vector.tensor_mul(outv[:], sig[:], ratio[:])
    nc.sync.dma_start(out[:], outv[:])


# ---------------------------------------------------------------------------
# General path (pm != 0): the proven baseline implementation, f32 throughout.
# ---------------------------------------------------------------------------

BC = 4
NCHUNK = BL // BC
UF_SCHED = [16, 32, 32, 32]
CHUNK_SIZES = [4, 4, 4, 4]
XP_BUFS = 2
WK_BUFS = 3
NE_P0 = 6553 // U
NE_U0 = 6553 - NE_P0 * U


def build_body(ctx, tc, x, params, utc, idc, out, pm_zero=False):
    nc = tc.nc
    pers = ctx.enter_context(tc.tile_pool(name="pers", bufs=1))
    xp = ctx.enter_context(tc.tile_pool(name="xp", bufs=XP_BUFS))
    wk = ctx.enter_context(tc.tile_pool(name="wk", bufs=WK_BUFS))
    psp = ctx.enter_context(tc.tile_pool(name="psp", bufs=2, space="PSUM"))
    ps1 = ctx.enter_context(tc.tile_pool(name="ps1", bufs=1, space="PSUM"))

    ut_t = pers.tile([P, P], F32)
    ones_t = pers.tile([P, P], F32)
    id_t = pers.tile([P, P], F32)
    gt = pers.tile([P, U], F32)
    ptile = pers.tile([P, 3], F32)
    nc.sync.dma_start(ptile[:], params[:])
    nc.gpsimd.memset(ones_t[:], 1.0)
    gti = pers.tile([P, U], mybir.dt.int32)
    nc.gpsimd.iota(gti[:], [[1, U]], base=0, channel_multiplier=U)
    nc.vector.tensor_copy(gt[:], gti[:])

    chunks = []
    o = 0
    for c in CHUNK_SIZES:
        chunks.append((o, c))
        o += c
    assert o == BL
    xts = []
    for ci, (bs, bc) in enumerate(chunks):
        xt = xp.tile([P, bc, U, N], F32, tag="xt")
        src = x[bs : bs + bc].rearrange("b (p u) n -> p b u n", p=P)
        if ci == 0:
            QU = U // 4
            for q in range(4):
                nc.sync.dma_start(
                    xt[:, :, q * QU : (q + 1) * QU, :],
                    src[:, :, q * QU : (q + 1) * QU, :],
                )
        else:
            HU = U // 2
            nc.sync.dma_start(xt[:, :, 0:HU, :], src[:, :, 0:HU, :])
            nc.sync.dma_start(xt[:, :, HU:U, :], src[:, :, HU:U, :])
        if ci == 0:
            nc.sync.dma_start(ut_t[:], utc[:])
            nc.sync.dma_start(id_t[:], idc[:])
        xts.append(xt)

    sv = scalar_prep(nc, pers, ptile)

    def sb(i, np_=P, p0=0):
        return sv[p0 : p0 + np_, i : i + 1]

    nf = pers.tile([P, U], F32)
    nc.vector.tensor_scalar_add(nf[:], gt[:], 1.0)
    zL = pers.tile([P, U], F32)
    nc.vector.tensor_scalar(zL[:], nf[:], sb(1), sb(2), ALU.mult, ALU.add)
    pvnL = pers.tile([P, U], F32)
    nc.vector.reciprocal(pvnL[:], zL[:])
    zR = pers.tile([P, U], F32)
    nc.vector.tensor_scalar(zR[:], gt[:], sb(3), sb(4), ALU.mult, ALU.add)
    pvnR = pers.tile([P, U], F32)
    nc.vector.reciprocal(pvnR[:], zR[:])
    lpvnL = pers.tile([P, U], F32)
    nc.scalar.activation(lpvnL[:], pvnL[:], AF.Ln)
    lpvnR = pers.tile([P, U], F32)
    nc.scalar.activation(lpvnR[:], pvnR[:], AF.Ln)
    kc2 = pers.tile([P, U], F32)
    nc.vector.tensor_add(kc2[:], lpvnL[:], lpvnR[:])

    nRf = pers.tile([P, U], F32)
    nc.vector.tensor_scalar(nRf[:], gt[:], -1.0, 8191.0, ALU.mult, ALU.add)
    gc = pers.tile([P, U], F32)
    nc.vector.tensor_scalar_max(gc[:], gt[:], 1.0)
    inv_n1 = pers.tile([P, U], F32)
    nc.vector.reciprocal(inv_n1[:], gc[:])
    nR1c = pers.tile([P, U], F32)
    nc.vector.tensor_scalar(nR1c[:], gt[:], -1.0, 8190.0, ALU.mult, ALU.add)
    nc.vector.tensor_scalar_max(nR1c[:], nR1c[:], 1.0)
    inv_nR1 = pers.tile([P, U], F32)
    nc.vector.reciprocal(inv_nR1[:], nR1c[:])
    inv_n = pers.tile([P, U], F32)
    nc.vector.reciprocal(inv_n[:], nf[:])
    inv_nR = pers.tile([P, U], F32)
    nRc = pers.tile([P, U], F32)
    nc.vector.tensor_scalar_max(nRc[:], nRf[:], 1.0)
    nc.vector.reciprocal(inv_nR[:], nRc[:])

    n_n1 = pers.tile([P, U], F32)
    nc.vector.tensor_mul(n_n1[:], nf[:], inv_n1[:])
    nR_nR1 = pers.tile([P, U], F32)
    nc.vector.tensor_mul(nR_nR1[:], nRf[:], inv_nR1[:])
    i_nn1 = pers.tile([P, U], F32)
    nc.vector.tensor_mul(i_nn1[:], inv_n[:], inv_n1[:])
    i_nRnR1 = pers.tile([P, U], F32)
    nc.vector.tensor_mul(i_nRnR1[:], inv_nR[:], inv_nR1[:])

    CBL = pers.tile([P, U], F32)
    nc.scalar.activation(CBL[:], n_n1[:], AF.Copy, scale=sb(7))
    CBR = pers.tile([P, U], F32)
    nc.scalar.activation(CBR[:], nR_nR1[:], AF.Copy, scale=sb(7))
    CA2L = pers.tile([P, U], F32)
    q1 = pers.tile([P, U], F32)
    nc.scalar.activation(q1[:], pvnL[:], AF.Copy, scale=sb(8))
    q2 = pers.tile([P, U], F32)
    nc.scalar.activation(q2[:], i_nn1[:], AF.Copy, scale=sb(9))
    nc.vector.tensor_add(CA2L[:], q1[:], q2[:])
    CA2R = pers.tile([P, U], F32)
    q1b = pers.tile([P, U], F32)
    nc.scalar.activation(q1b[:], pvnR[:], AF.Copy, scale=sb(8))
    q2b = pers.tile([P, U], F32)
    nc.scalar.activation(q2b[:], i_nRnR1[:], AF.Copy, scale=sb(9))
    nc.vector.tensor_add(CA2R[:], q1b[:], q2b[:])
    CAL = pers.tile([P, U], F32)
    nc.scalar.activation(CAL[:], pvnL[:], AF.Copy, scale=sb(10))
    CAR = pers.tile([P, U], F32)
    nc.scalar.activation(CAR[:], pvnR[:], AF.Copy, scale=sb(10))
    Cc = pers.tile([P, U], F32)
    p12 = pers.tile([P, U], F32)
    nc.vector.tensor_add(p12[:], pvnL[:], pvnR[:])
    cc1 = pers.tile([P, U], F32)
    nc.scalar.activation(cc1[:], p12[:], AF.Copy, scale=sb(11))
    cct = pers.tile([P, U], F32)
    nc.vector.tensor_scalar(cct[:], kc2[:], 0.5, sb(12), ALU.mult, ALU.add)
    nc.vector.tensor_add(Cc[:], cc1[:], cct[:])
    mlo = pers.tile([P, U], F32)
    nc.vector.tensor_scalar(mlo[:], gt[:], 14.5, NEG, ALU.is_lt, ALU.mult)
    mhi = pers.tile([P, U], F32)
    nc.vector.tensor_scalar(mhi[:], gt[:], 8174.5, NEG, ALU.is_ge, ALU.mult)
    nc.vector.tensor_add(Cc[:], Cc[:], mlo[:])
    nc.vector.tensor_add(Cc[:], Cc[:], mhi[:])
    nemask = pers.tile([P, U], F32)
    nc.vector.tensor_scalar(nemask[:], gt[:], 6552.5, None, ALU.is_ge)

    bund = pers.tile([P, 80], F32)
    zeros = pers.tile([P, max(CHUNK_SIZES) * U], F32)
    nc.gpsimd.memset(zeros[:], 0.0)

    Tall = ps1.tile([BL, 5, P], F32)
    Tm = Tall[:, 0, :]
    Tz = Tall[:, 1, :]
    Te = Tall[:, 2, :]
    Ta = Tall[:, 3, :]
    Tb = Tall[:, 4, :]
    M16 = pers.tile([BL, 1], F32)
    d = pers.tile([BL, P], F32)
    w = pers.tile([BL, P], F32)
    bfW = pers.tile([BL, 1], F32)
    sig = pers.tile([BL, 1], F32)

    def trace_bfw():
        nc.tensor.transpose(Ta, bund[:, 48 : 48 + BL], id_t[:])
        nc.tensor.transpose(Tb, bund[:, 64 : 64 + BL], id_t[:])
        At16 = Ta[:, 0:1]
        Bt16 = Tb[:, 0:1]
        t1 = pers.tile([BL, 1], F32)
        nc.scalar.activation(t1[:], At16, AF.Square, scale=1.0 / 32.0)
        v2 = pers.tile([BL, 1], F32)
        nc.vector.tensor_scalar_mul(v2[:], t1[:], 1.0 / 8192.0)
        vW = pers.tile([BL, 1], F32)
        nc.vector.scalar_tensor_tensor(
            vW[:], Bt16, 1.0 / 1024.0, v2[:], ALU.mult, ALU.subtract
        )
        nc.vector.tensor_scalar(vW[:], vW[:], 1.0 / 8191.0, 1.0e-8, ALU.mult, ALU.max)
        term1 = pers.tile([BL, 1], F32)
        nc.vector.tensor_scalar_mul(term1[:], vW[:], sb(17, BL))
        term2 = pers.tile([BL, 1], F32)
        nc.vector.tensor_scalar_mul(term2[:], t1[:], sb(18, BL))
        uu = pers.tile([BL, 1], F32)
        nc.scalar.activation(uu[:], At16, AF.Identity, bias=sb(6, BL), scale=sb(5, BL))
        u2 = pers.tile([BL, 1], F32)
        nc.scalar.activation(u2[:], uu[:], AF.Square)
        term3 = pers.tile([BL, 1], F32)
        nc.vector.tensor_scalar_mul(term3[:], u2[:], sb(13, BL))
        tsum = pers.tile([BL, 1], F32)
        nc.vector.tensor_add(tsum[:], term1[:], term2[:])
        nc.vector.tensor_sub(tsum[:], tsum[:], term3[:])
        nc.vector.tensor_scalar(bfW[:], tsum[:], -0.5, sb(19, BL), ALU.mult, ALU.add)

    def trace_maxw():
        nc.tensor.transpose(Tm, bund[:, 0:BL], id_t[:])
        nc.vector.tensor_reduce(M16[:], Tm, AX.X, ALU.min, negate=True)
        nc.vector.tensor_scalar(d[:], Tm, -1.0, M16[:], ALU.mult, ALU.subtract)
        nc.scalar.activation(w[:], d[:], AF.Exp)
        sigin = pers.tile([BL, 1], F32)
        nc.vector.tensor_sub(sigin[:], M16[:], bfW[:])
        nc.scalar.activation(sig[:], sigin[:], AF.Sigmoid)

    xhs = [None] * len(chunks)

    def ufof(ci):
        return UF_SCHED[ci] if UF_SCHED is not None else 32

    def trace_fold(ci):
        uf = ufof(ci)
        bc = chunks[ci][1]
        if uf > 0:
            xh = wk.tile([P, bc, uf, 16], F32, tag="xh")
            nc.gpsimd.tensor_add(
                xh[:], xts[ci][:, :, 0:uf, 0:16], xts[ci][:, :, 0:uf, 16:32]
            )
            xhs[ci] = xh

    trace_fold(0)
    for ci, (bs, bc) in enumerate(chunks):
        xt = xts[ci]
        last = ci == len(chunks) - 1
        if not last:
            trace_fold(ci + 1)

        uf = ufof(ci)
        sr = wk.tile([P, bc, U], F32)
        if uf > 0:
            if ci == 0 and uf == 16:
                for q in range(1, 4):
                    nc.vector.tensor_reduce(
                        sr[:, :, q * 16 : (q + 1) * 16],
                        xt[:, :, q * 16 : (q + 1) * 16, :],
                        AX.X,
                        ALU.add,
                    )
            elif uf < U:
                nc.vector.tensor_reduce(
                    sr[:, :, uf:U], xt[:, :, uf:U, :], AX.X, ALU.add
                )
            nc.vector.tensor_reduce(sr[:, :, 0:uf], xhs[ci][:], AX.X, ALU.add)
        else:
            HU = U // 2
            nc.vector.tensor_reduce(
                sr[:, :, 0:HU], xt[:, :, 0:HU, :], AX.X, ALU.add
            )
            nc.vector.tensor_reduce(
                sr[:, :, HU:U], xt[:, :, HU:U, :], AX.X, ALU.add
            )
        sq = wk.tile([P, bc, U], F32)
        nc.scalar.activation(sq[:], sr[:], AF.Square)

        A = wk.tile([P, bc, U], F32)
        nc.vector.tensor_tensor_scan(
            A[:].rearrange("p b u -> p (b u)"),
            sr[:].rearrange("p b u -> p (b u)"),
            zeros[:, 0 : bc * U],
            0.0,
            ALU.add,
            ALU.add,
        )
        Bt_ = wk.tile([P, bc, U], F32)
        nc.vector.tensor_tensor_scan(
            Bt_[:].rearrange("p b u -> p (b u)"),
            sq[:].rearrange("p b u -> p (b u)"),
            zeros[:, 0 : bc * U],
            0.0,
            ALU.add,
            ALU.add,
        )

        rv = wk.tile([P, 2 * bc], F32)
        nc.gpsimd.memset(rv[:, 0:1], 0.0)
        nc.gpsimd.memset(rv[:, bc : bc + 1], 0.0)
        nc.vector.tensor_copy(rv[:, 1:bc], A[:, 0 : bc - 1, U - 1])
        nc.vector.tensor_copy(rv[:, bc + 1 : 2 * bc], Bt_[:, 0 : bc - 1, U - 1])
        ct = wk.tile([P, 2 * bc], F32)
        nc.vector.tensor_sub(ct[:, 0:bc], A[:, :, U - 1], rv[:, 0:bc])
        nc.vector.tensor_sub(ct[:, bc : 2 * bc], Bt_[:, :, U - 1], rv[:, bc : 2 * bc])
        g_ps = psp.tile([P, 2 * bc], F32)
        nc.tensor.matmul(g_ps[:], ut_t[:], ct[:])
        tot_ps = psp.tile([P, 2 * bc], F32)
        nc.tensor.matmul(tot_ps[:], ones_t[:], ct[:])
        off = wk.tile([P, 2 * bc], F32)
        nc.vector.tensor_sub(off[:], g_ps[:], rv[:])

        offA_b = off[:, 0:bc].unsqueeze(2).broadcast_to([P, bc, U])
        offB_b = off[:, bc : 2 * bc].unsqueeze(2).broadcast_to([P, bc, U])
        nc.vector.tensor_add(A[:], A[:], offA_b)
        nc.vector.tensor_add(Bt_[:], Bt_[:], offB_b)

        nc.scalar.copy(bund[:, 48 + bs : 48 + bs + bc], tot_ps[:, 0:bc])
        nc.scalar.copy(bund[:, 64 + bs : 64 + bs + bc], tot_ps[:, bc : 2 * bc])
        if last:
            trace_bfw()
        At_b = (
            bund[:, 48 + bs : 48 + bs + bc].unsqueeze(2).broadcast_to([P, bc, U])
        )
        Btot_b = (
            bund[:, 64 + bs : 64 + bs + bc].unsqueeze(2).broadcast_to([P, bc, U])
        )
        AR = wk.tile([P, bc, U], F32)
        nc.gpsimd.tensor_sub(AR[:], At_b, A[:])
        BR = wk.tile([P, bc, U], F32)
        nc.gpsimd.tensor_sub(BR[:], Btot_b, Bt_[:])

        A2 = wk.tile([P, bc, U], F32)
        nc.scalar.activation(A2[:], A[:], AF.Square)
        AR2 = wk.tile([P, bc, U], F32)
        nc.scalar.activation(AR2[:], AR[:], AF.Square)

        def cb(t):
            return t[:].unsqueeze(1).broadcast_to([P, bc, U])

        bf = wk.tile([P, bc, U], F32)
        p2 = wk.tile([P, bc, U], F32)
        p3 = wk.tile([P, bc, U], F32)
        p5 = wk.tile([P, bc, U], F32)
        p6 = wk.tile([P, bc, U], F32)
        nc.vector.tensor_mul(p2[:], A2[:], cb(CA2L))
        nc.vector.tensor_mul(p3[:], Bt_[:], cb(CBL))
        nc.vector.tensor_mul(p5[:], AR2[:], cb(CA2R))
        (nc.vector if last else nc.gpsimd).tensor_mul(p6[:], BR[:], cb(CBR))
        p1 = wk.tile([P, bc, U], F32)
        p4 = wk.tile([P, bc, U], F32)
        nc.vector.tensor_mul(p1[:], A[:], cb(CAL))
        nc.vector.tensor_mul(p4[:], AR[:], cb(CAR))
        nc.gpsimd.tensor_add(p1[:], p1[:], p2[:])
        nc.vector.tensor_add(p3[:], p3[:], p4[:])
        nc.gpsimd.tensor_add(p5[:], p5[:], p6[:])
        nc.gpsimd.tensor_add(p1[:], p1[:], cb(Cc))
        nc.vector.tensor_add(p3[:], p3[:], p5[:])
        nc.vector.tensor_add(bf[:], p1[:], p3[:])

        nc.vector.tensor_reduce(
            bund[:, bs : bs + bc], bf[:], AX.X, ALU.max, negate=True
        )
        if last:
            trace_maxw()
        e = wk.tile([P, bc, U], F32)
        for b in range(bc):
            nc.scalar.activation(
                e[:, b, :],
                bf[:, b, :],
                AF.Exp,
                bias=bund[:, bs + b : bs + b + 1],
                accum_out=bund[:, 16 + bs + b : 17 + bs + b],
            )
        en = wk.tile([P, bc, U], F32)
        (nc.vector if last else nc.gpsimd).tensor_mul(en[:], e[:], cb(nemask))
        nc.vector.tensor_reduce(
            bund[:, 32 + bs : 32 + bs + bc], en[:], AX.X, ALU.add
        )

    nc.tensor.transpose(Tz, bund[:, 16 : 16 + BL], id_t[:])
    nc.tensor.transpose(Te, bund[:, 32 : 32 + BL], id_t[:])
    wz = pers.tile([BL, P], F32)
    Zb = pers.tile([BL, 1], F32)
    nc.vector.scalar_tensor_tensor(
        wz[:], w[:], 1.0, Tz, ALU.mult, ALU.mult, accum_out=Zb[:]
    )
    wn = pers.tile([BL, P], F32)
    Nb = pers.tile([BL, 1], F32)
    nc.vector.scalar_tensor_tensor(
        wn[:], w[:], 1.0, Te, ALU.mult, ALU.mult, accum_out=Nb[:]
    )
    invZ = pers.tile([BL, 1], F32)
    nc.vector.reciprocal(invZ[:], Zb[:])
    ratio = pers.tile([BL, 1], F32)
    nc.vector.tensor_mul(ratio[:], Nb[:], invZ[:])
    outv = pers.tile([BL, 1], F32)
    nc.vector.tensor_mul(outv[:], sig[:], ratio[:])
    nc.sync.dma_start(out[:], outv[:])


def host_consts():
    ut = np.triu(np.ones((P, P), np.float32), 1)
    ident = np.eye(P, dtype=np.float32)
    return ut, ident


def split_multi_waits(nc):
    """Walrus in this container allows one sync wait per instruction; move
    extra waits onto same-engine NOPs inserted immediately before."""
    import bass_rust

    nid = [0]
    for f in nc.m.functions:
        for b in f.blocks:
            insts = b.instructions
            i = 0
            while i < len(insts):
                ins = insts[i]
                si = ins.sync_info
                if si is not None and si.on_wait is not None and len(si.on_wait) > 1:
                    waits = list(si.on_wait)
                    for w in waits[:-1]:
                        nop = mybir.InstNoOp(
                            name=f"I-waitsplit-{nid[0]}", ins=[], outs=[]
                        )
                        nid[0] += 1
                        nop.engine = ins.engine
                        nop.sync_info = bass_rust.SyncInfo(
                            on_wait=[w], on_update=[]
                        )
                        insts.insert(i, nop)
                        i += 1
                    si.on_wait = waits[-1:]
                i += 1


_NC_CACHE = {}


def build_nc(split=True, reps=1, pm_zero=False):
    global _NC_CACHE
    key = (split, reps, pm_zero)
    if key in _NC_CACHE:
        return _NC_CACHE[key]
    nc = bass.Bass()
    x = nc.declare_dram_parameter("x", [BL, T, N], F32, isOutput=False)
    params = nc.declare_dram_parameter("params", [P, 3], F32, isOutput=False)
    utc = nc.declare_dram_parameter("utc", [P, P], F32, isOutput=False)
    idc = nc.declare_dram_parameter("idc", [P, P], F32, isOutput=False)
    out = nc.declare_dram_parameter("out", [BL, 1], F32, isOutput=True)
    with tile.TileContext(nc) as tc:
        for _ in range(reps):
            with ExitStack() as ctx:
                if pm_zero:
                    build_body_pm0(
                        ctx, tc, x[:], params[:], utc[:], idc[:], out[:]
                    )
                else:
                    build_body(
                        ctx, tc, x[:], params[:], utc[:], idc[:], out[:]
                    )
    if split:
        split_multi_waits(nc)
    _NC_CACHE[key] = nc
    return nc


def make_in_maps(x, prior_mean, prior_var, noise_var):
    x = np.ascontiguousarray(np.asarray(x, dtype=np.float32))
    params = np.tile(
        np.array(
            [[float(prior_mean[0]), float(prior_var[0]), float(noise_var[0])]],
            dtype=np.float32,
        ),
        (P, 1),
    )
    ut, ident = host_consts()
    in_maps = []
    for c in range(NCORES):
        in_maps.append(
            {
                "x": x[c * BL : (c + 1) * BL],
                "params": params,
                "utc": ut,
                "idc": ident,
            }
        )
    return in_maps


def kernel(x, prior_mean, prior_var, noise_var):
    from concourse.bass_utils import run_bass_kernel_spmd

    in_maps = make_in_maps(x, prior_mean, prior_var, noise_var)
    nc = build_nc(pm_zero=(float(np.asarray(prior_mean).reshape(-1)[0]) == 0.0))
    res = run_bass_kernel_spmd(nc, in_maps, list(range(NCORES)))
    outs = [np.asarray(res.results[c]["out"]).reshape(BL) for c in range(NCORES)]
    return np.concatenate(outs).astype(np.float32)


# revision 45
# speedup vs baseline: 1.4712x; 1.1830x over previous
"""Trainium2 Bass kernel for BayesianChangePointDetector (segment_reduce).

Contract: kernel(**inputs) takes FULL inputs (x:[128,8192,32] f32, plus 3
scalar prior params) and returns the FULL [128] f32 output. Internally the
batch dim is sharded across 8 NeuronCores (16 rows each, pure data parallel,
no collectives), each core runs the same Bass/Tile program, and the host
concatenates the 8 per-core [16] outputs.

Fast path (pm == 0, the shipped input): x is cast f32->fp16 during the SWDGE
DMA (halves the charged DMA time), the N=32 fold is a fp16 binary add-tree
(32->16->8->4 on DVE in 2x perf mode, 4->2->1 on Pool), prefix sums are
per-row f32 scans with a triangular-ones carry matmul on PE, and the Bayes
factor is assembled as bf'' = CA2L*A^2 + CA2R*(At-A)^2 + SB*B + CBR2*Btot[b]
+ Cc_var with all large per-row constants moved into the final sigmoid path,
so bf'' is O(10) and exp needs no max-shift. A^2 and (At-A)^2 come from ACT
Square activations whose bias folds in the carry offsets.
"""

import sys

if "/opt/trn_rl_repo" not in sys.path:
    sys.path.insert(0, "/opt/trn_rl_repo")

import math
from contextlib import ExitStack

import numpy as np

import concourse.bass as bass
import concourse.tile as tile
from concourse import mybir

F32 = mybir.dt.float32
F16 = mybir.dt.float16
AF = mybir.ActivationFunctionType
ALU = mybir.AluOpType
AX = mybir.AxisListType

B, T, N = 128, 8192, 32
NCORES = 8
BL = B // NCORES  # 16 batch rows per core
P = 128           # partitions = t-blocks
U = T // P        # 64 t's per partition
NS = 32           # scalar-slot count
NEG = -1.0e30
NEG16 = -60000.0  # mask value that survives an fp16 round-trip

# fast-path chunking: small last chunk to shrink the post-DMA tail
CHUNKS_PM0 = [(0, 5), (5, 5), (10, 4), (14, 2)]  # DMA transfer groups
# compute chunks (must tile the DMA groups); None -> same as CHUNKS_PM0
COMP_PM0 = None
TTR_FUSE = False   # fuse final bf add + per-b max via tensor_tensor_reduce
SPLIT_FOLDS_ALL = False  # u-half folds for all chunks (vs chunks 0-1 only)

# near-end threshold: candidates g >= 6553 (g = 64p+u); p=102 partial (u>=25),
# p>=103 fully near-end.
NE_P = 102
NE_U = 25
# valid candidates: g in [15, 8175)
LO_THR = 14.5
HI_THR = 8174.5


def scalar_prep(nc, pers, ptile):
    """Per-partition scalar slots, identical math to the baseline kernel."""
    sv = pers.tile([P, NS], F32)
    tmp = pers.tile([P, 8], F32)

    def s(i):
        return sv[:, i : i + 1]

    def tm(i):
        return tmp[:, i : i + 1]

    # softplus(x) = ln(1 + exp(x))
    nc.scalar.activation(tm(0), ptile[:, 1:2], AF.Exp)
    nc.vector.tensor_scalar_add(tm(0), tm(0), 1.0)
    nc.scalar.activation(s(20), tm(0), AF.Ln)
    nc.scalar.activation(tm(1), ptile[:, 2:3], AF.Exp)
    nc.vector.tensor_scalar_add(tm(1), tm(1), 1.0)
    nc.scalar.activation(s(21), tm(1), AF.Ln)
    nc.vector.tensor_copy(s(0), ptile[:, 0:1])
    nc.vector.reciprocal(s(1), s(21))
    nc.vector.reciprocal(s(2), s(20))
    nc.vector.tensor_scalar_mul(s(3), s(1), -1.0)
    nc.vector.tensor_scalar(s(4), s(1), 8191.0, s(2), ALU.mult, ALU.add)
    nc.vector.tensor_scalar_mul(s(5), s(1), 1.0 / 32.0)
    nc.vector.tensor_mul(s(6), s(0), s(2))
    nc.vector.tensor_scalar_mul(s(7), s(1), -0.5 / 1024.0)
    nc.vector.tensor_scalar_mul(s(9), s(1), 0.5 / 1024.0)
    nc.vector.tensor_mul(tm(0), s(5), s(5))
    nc.vector.tensor_scalar_mul(s(8), tm(0), 0.5)
    nc.vector.tensor_mul(s(10), s(6), s(5))
    nc.vector.tensor_mul(tm(1), s(6), s(6))
    nc.vector.tensor_scalar_mul(s(11), tm(1), 0.5)
    nc.scalar.activation(s(14), s(21), AF.Ln, scale=2.0 * math.pi)
    nc.scalar.activation(s(15), s(20), AF.Ln)
    nc.vector.tensor_scalar_mul(s(17), s(1), 8192.0)
    nc.vector.tensor_scalar(tm(2), s(1), 8192.0, s(2), ALU.mult, ALU.add)
    nc.vector.reciprocal(s(13), tm(2))
    nc.scalar.activation(s(16), s(13), AF.Ln)
    nc.vector.tensor_scalar_mul(s(18), s(1), 1.0 / 8192.0)
    nc.vector.tensor_mul(tm(3), s(0), s(0))
    nc.vector.tensor_mul(s(22), tm(3), s(2))
    nc.vector.tensor_scalar_mul(s(23), s(14), -4096.0)
    nc.vector.tensor_sub(tm(4), s(23), s(15))
    nc.vector.tensor_sub(s(12), tm(4), s(22))
    nc.vector.tensor_sub(tm(5), s(16), s(15))
    nc.vector.tensor_scalar_mul(tm(5), tm(5), 0.5)
    nc.vector.tensor_add(tm(6), s(23), tm(5))
    nc.vector.tensor_scalar_mul(tm(7), s(22), -0.5)
    nc.vector.tensor_add(s(19), tm(6), tm(7))
    return sv


def build_body_pm0(ctx, tc, x, coef16, coef32, utc, idc, out):
    nc = tc.nc
    pers = ctx.enter_context(tc.tile_pool(name="pers", bufs=1))
    xp = ctx.enter_context(tc.tile_pool(name="xp", bufs=3))
    wk = ctx.enter_context(tc.tile_pool(name="wk", bufs=3))
    psp = ctx.enter_context(tc.tile_pool(name="psp", bufs=2, space="PSUM"))
    ps1 = ctx.enter_context(tc.tile_pool(name="ps1", bufs=1, space="PSUM"))

    groups = CHUNKS_PM0
    chunks = COMP_PM0 if COMP_PM0 is not None else CHUNKS_PM0
    nch = len(chunks)
    ngr = len(groups)

    # ---------- consts + early DMAs ----------
    ut_t = pers.tile([P, P], F32)
    ones_t = pers.tile([P, P], F32)
    id_t = pers.tile([P, P], F32)
    bund = pers.tile([P, 80], F32)  # [0:16) rmax | [16:32) Zp | [32:48) En | [48:64) At | [64:80) Btot
    zeros = pers.tile([P, U], F32)

    xts = [None] * ngr
    next_g = [0]

    def issue_dma(gi):
        gs, gc_ = groups[gi]
        xt = xp.tile([P, gc_, U, N], F16, tag="xt")
        src = x[gs : gs + gc_].rearrange("b (p u) n -> p b u n", p=P)
        npieces = 2 if gi <= 1 else 1
        UQ = U // npieces
        for q in range(npieces):
            nc.gpsimd.dma_start(
                xt[:, :, q * UQ : (q + 1) * UQ, :],
                src[:, :, q * UQ : (q + 1) * UQ, :],
            )
        xts[gi] = xt

    def emit_next_dma():
        if next_g[0] < ngr:
            issue_dma(next_g[0])
            next_g[0] += 1

    def xt_view(ci):
        bs, bc = chunks[ci]
        for gi, (gs, gc_) in enumerate(groups):
            if gs <= bs and bs + bc <= gs + gc_:
                return xts[gi][:, bs - gs : bs - gs + bc]
        raise AssertionError("compute chunk not inside a DMA group")

    # Pool stream: x DMAs first, then memsets
    emit_next_dma()
    emit_next_dma()
    nc.gpsimd.memset(ones_t[:], 1.0)
    nc.gpsimd.memset(zeros[:], 0.0)

    # host-precomputed coefficient vectors + scalar slots via HWDGE
    c16 = pers.tile([P, 3 * U], F16)
    nc.sync.dma_start(c16[:], coef16[:])
    c32 = pers.tile([P, 2 * U + NS], F32)
    nc.sync.dma_start(c32[:], coef32[:])
    nc.sync.dma_start(ut_t[:], utc[:])
    nc.sync.dma_start(id_t[:], idc[:])
    CA2Lh = c16[:, 0:U]
    CA2Rh = c16[:, U : 2 * U]
    Ccvh = c16[:, 2 * U : 3 * U]
    SBt = c32[:, 0:U]
    CBR2 = c32[:, U : 2 * U]

    def sb(i, np_=P):
        return c32[0:np_, 2 * U + i : 2 * U + i + 1]

    # ---------- finale tiles ----------
    Tall = ps1.tile([BL, 5, P], F32)
    Tm = Tall[:, 0, :]
    Tz = Tall[:, 1, :]
    Te = Tall[:, 2, :]
    Ta = Tall[:, 3, :]
    Tb = Tall[:, 4, :]
    M16 = pers.tile([BL, 1], F32)
    bfW = pers.tile([BL, 1], F32)
    sig = pers.tile([BL, 1], F32)

    def cbc(ap, bc):
        return ap.unsqueeze(1).broadcast_to([P, bc, U])

    # ---------- per-chunk pipeline (software-pipelined stage emission) ----------
    st = [dict() for _ in range(nch)]
    HU = U // 2

    def S1_folds(ci):
        # u-piece-split fp16 fold tree: DVE 32->16->8->4 (2x), Pool 4->2->1
        bs, bc = chunks[ci]
        xt = xt_view(ci)
        h1 = wk.tile([P, bc, U, 16], F16, tag="h1")
        h2 = wk.tile([P, bc, U, 8], F16, tag="h2")
        h3 = wk.tile([P, bc, U, 4], F16, tag="h3")
        h4 = wk.tile([P, bc, U, 2], F16, tag="h4")
        sr = wk.tile([P, bc, U], F32, tag="sr")
        npieces = 2 if (ci <= 1 or SPLIT_FOLDS_ALL) else 1
        UQ = U // npieces
        spans = [(q * UQ, (q + 1) * UQ) for q in range(npieces)]
        for lo, hi in spans:
            nc.vector.tensor_add(
                h1[:, :, lo:hi, :], xt[:, :, lo:hi, 0:16], xt[:, :, lo:hi, 16:32]
            )
            nc.vector.tensor_add(
                h2[:, :, lo:hi, :], h1[:, :, lo:hi, 0:8], h1[:, :, lo:hi, 8:16]
            )
            nc.vector.tensor_add(
                h3[:, :, lo:hi, :], h2[:, :, lo:hi, 0:4], h2[:, :, lo:hi, 4:8]
            )
            nc.gpsimd.tensor_add(
                h4[:, :, lo:hi, :], h3[:, :, lo:hi, 0:2], h3[:, :, lo:hi, 2:4]
            )
            nc.gpsimd.tensor_add(
                sr[:, :, lo:hi], h4[:, :, lo:hi, 0:1], h4[:, :, lo:hi, 1:2]
            )
        st[ci]["sr"] = sr

    def S2_scans(ci):
        bs, bc = chunks[ci]
        sr = st[ci]["sr"]
        sq = wk.tile([P, bc, U], F32, tag="sq")
        nc.scalar.activation(sq[:], sr[:], AF.Square)
        A = wk.tile([P, bc, U], F32, tag="A")
        Bt_ = wk.tile([P, bc, U], F32, tag="B")
        for b in range(bc):
            nc.vector.tensor_tensor_scan(
                A[:, b, :], sr[:, b, :], zeros[:], 0.0, ALU.add, ALU.add
            )
        for b in range(bc):
            nc.vector.tensor_tensor_scan(
                Bt_[:, b, :], sq[:, b, :], zeros[:], 0.0, ALU.add, ALU.add
            )
        st[ci]["A"] = A
        st[ci]["B"] = Bt_

    def S3_carry(ci):
        # separate A/B carry paths so the A side unblocks early
        bs, bc = chunks[ci]
        A, Bt_ = st[ci]["A"], st[ci]["B"]
        g_ps = psp.tile([P, 2 * bc], F32, tag="gps")
        t_ps = psp.tile([P, 2 * bc], F32, tag="tot")
        ctA = wk.tile([P, bc], F32, tag="ctA")
        nc.scalar.copy(ctA[:], A[:, :, U - 1])
        nc.tensor.matmul(g_ps[:, 0:bc], ut_t[:], ctA[:])
        nc.tensor.matmul(t_ps[:, 0:bc], ones_t[:], ctA[:])
        offsA = wk.tile([P, bc], F32, tag="offsA")
        nc.scalar.copy(offsA[:], g_ps[:, 0:bc])
        nc.scalar.copy(bund[:, 48 + bs : 48 + bs + bc], t_ps[:, 0:bc])
        ctB = wk.tile([P, bc], F32, tag="ctB")
        nc.scalar.copy(ctB[:], Bt_[:, :, U - 1])
        nc.tensor.matmul(g_ps[:, bc : 2 * bc], ut_t[:], ctB[:])
        nc.tensor.matmul(t_ps[:, bc : 2 * bc], ones_t[:], ctB[:])
        offsB = wk.tile([P, bc], F32, tag="offsB")
        nc.scalar.copy(offsB[:], g_ps[:, bc : 2 * bc])
        nc.scalar.copy(bund[:, 64 + bs : 64 + bs + bc], t_ps[:, bc : 2 * bc])
        st[ci]["offsA"] = offsA
        st[ci]["offsB"] = offsB

    def S4_apply(ci):
        bs, bc = chunks[ci]
        A, Bt_ = st[ci]["A"], st[ci]["B"]
        offA_b = st[ci]["offsA"][:].unsqueeze(2).broadcast_to([P, bc, U])
        nc.gpsimd.tensor_add(A[:], A[:], offA_b)
        offB_b = st[ci]["offsB"][:].unsqueeze(2).broadcast_to([P, bc, U])
        nc.gpsimd.tensor_add(Bt_[:], Bt_[:], offB_b)
        m3h = wk.tile([P, bc, U], F16, tag="m3h")
        nc.gpsimd.tensor_mul(m3h[:], Bt_[:], cbc(SBt, bc))  # m3, fp16 out
        st[ci]["m3"] = m3h

    def S5_squares(ci):
        bs, bc = chunks[ci]
        A = st[ci]["A"]
        ARt = wk.tile([P, bc, U], F32, tag="ARt")
        At_b = bund[:, 48 + bs : 48 + bs + bc].unsqueeze(2).broadcast_to([P, bc, U])
        nc.gpsimd.tensor_sub(ARt[:], At_b, A[:])
        A2 = wk.tile([P, bc, U], F16, tag="A2")
        nc.scalar.activation(A2[:], A[:], AF.Square, scale=1.0 / 64.0)
        AR2 = wk.tile([P, bc, U], F16, tag="AR2")
        nc.scalar.activation(AR2[:], ARt[:], AF.Square, scale=1.0 / 64.0)
        nc.vector.tensor_mul(A2[:], A2[:], cbc(CA2Lh, bc))
        nc.vector.tensor_mul(AR2[:], AR2[:], cbc(CA2Rh, bc))
        nc.vector.tensor_add(A2[:], A2[:], AR2[:])
        nc.vector.tensor_add(A2[:], A2[:], cbc(Ccvh, bc))  # + Cc_var, fp16 2x
        st[ci]["t1"] = A2

    def S6_bf(ci):
        bs, bc = chunks[ci]
        m3h = st[ci]["m3"]
        # Kb = CBR2 * Btot[b] on ACT (per-partition-scalar scale), fp16 out
        Kb = wk.tile([P, bc, U], F16, tag="Kc")
        for b in range(bc):
            nc.scalar.activation(
                Kb[:, b, :], CBR2, AF.Copy,
                scale=bund[:, 64 + bs + b : 65 + bs + b],
            )
        nc.vector.tensor_add(m3h[:], m3h[:], Kb[:])
        st[ci]["bf"] = m3h

    def S7_exp(ci):
        # final bf add + per-b max (fused via tensor_tensor_reduce when on;
        # bund rmax is stored positive either way)
        bs, bc = chunks[ci]
        bf = st[ci]["bf"]
        t1 = st[ci]["t1"]
        if TTR_FUSE:
            for b in range(bc):
                nc.vector.tensor_tensor_reduce(
                    bf[:, b, :], bf[:, b, :], t1[:, b, :], 1.0, NEG,
                    ALU.add, ALU.max, bund[:, bs + b : bs + b + 1],
                )
        else:
            nc.vector.tensor_add(bf[:], bf[:], t1[:])
            nc.vector.tensor_reduce(
                bund[:, bs : bs + bc], bf[:], AX.X, ALU.max
            )
        e = wk.tile([P, bc, U], F32, tag="e")
        nc.scalar.activation(e[:], bf[:], AF.Exp)
        st[ci]["e"] = e

    def S8_sums(ci):
        # Zp = head + tail partial sums; tail (u>=NE_U) doubles as the
        # near-end partial (only row p=102 is consumed by the finale)
        bs, bc = chunks[ci]
        e = st[ci]["e"]
        nc.vector.tensor_reduce(
            bund[:, 32 + bs : 32 + bs + bc], e[:, :, NE_U:U], AX.X, ALU.add
        )
        zh = wk.tile([P, bc], F32, tag="zh")
        nc.vector.tensor_reduce(zh[:], e[:, :, 0:NE_U], AX.X, ALU.add)
        nc.vector.tensor_add(
            bund[:, 16 + bs : 16 + bs + bc], zh[:],
            bund[:, 32 + bs : 32 + bs + bc],
        )

    S1_folds(0)
    S2_scans(0)
    emit_next_dma()
    S3_carry(0)
    S1_folds(1)
    emit_next_dma()
    for ci in range(nch):
        S4_apply(ci)
        S5_squares(ci)
        if ci >= 1:
            S8_sums(ci - 1)
        if ci + 1 < nch:
            S2_scans(ci + 1)
        S6_bf(ci)
        if ci + 1 < nch:
            S3_carry(ci + 1)
        if ci + 2 < nch:
            S1_folds(ci + 2)
        S7_exp(ci)
    S8_sums(nch - 1)

    # ---------- finale ----------
    # whole-window log marginal (needs At/Btot of all rows)
    nc.tensor.transpose(Ta, bund[:, 48 : 48 + BL], id_t[:])
    nc.tensor.transpose(Tb, bund[:, 64 : 64 + BL], id_t[:])
    At16 = Ta[:, 0:1]
    Bt16 = Tb[:, 0:1]
    t1 = pers.tile([BL, 1], F32)
    nc.scalar.activation(t1[:], At16, AF.Square, scale=1.0 / 32.0)
    v2 = pers.tile([BL, 1], F32)
    nc.vector.tensor_scalar_mul(v2[:], t1[:], 1.0 / 8192.0)
    vW = pers.tile([BL, 1], F32)
    nc.vector.scalar_tensor_tensor(
        vW[:], Bt16, 1.0 / 1024.0, v2[:], ALU.mult, ALU.subtract
    )
    nc.vector.tensor_scalar(vW[:], vW[:], 1.0 / 8191.0, 1.0e-8, ALU.mult, ALU.max)
    term1 = pers.tile([BL, 1], F32)
    nc.vector.tensor_scalar_mul(term1[:], vW[:], sb(17, BL))
    term2 = pers.tile([BL, 1], F32)
    nc.vector.tensor_scalar_mul(term2[:], t1[:], sb(18, BL))
    uu = pers.tile([BL, 1], F32)
    nc.scalar.activation(uu[:], At16, AF.Identity, bias=sb(6, BL), scale=sb(5, BL))
    u2 = pers.tile([BL, 1], F32)
    nc.scalar.activation(u2[:], uu[:], AF.Square)
    term3 = pers.tile([BL, 1], F32)
    nc.vector.tensor_scalar_mul(term3[:], u2[:], sb(13, BL))
    tsum = pers.tile([BL, 1], F32)
    nc.vector.tensor_add(tsum[:], term1[:], term2[:])
    nc.vector.tensor_sub(tsum[:], tsum[:], term3[:])
    nc.vector.tensor_scalar(bfW[:], tsum[:], -0.5, sb(19, BL), ALU.mult, ALU.add)
    # cW = sc - (kq/2)*Btot - bfW ; true max = M16 + cW
    cw1 = pers.tile([BL, 1], F32)
    nc.vector.scalar_tensor_tensor(
        cw1[:], Bt16, sb(9, BL), bfW[:], ALU.mult, ALU.add
    )
    cW = pers.tile([BL, 1], F32)
    nc.vector.tensor_scalar(cW[:], cw1[:], -1.0, sb(12, BL), ALU.mult, ALU.add)

    nc.tensor.transpose(Tm, bund[:, 0:BL], id_t[:])
    nc.vector.tensor_reduce(M16[:], Tm, AX.X, ALU.max)
    sigin = pers.tile([BL, 1], F32)
    nc.vector.tensor_add(sigin[:], M16[:], cW[:])
    nc.scalar.activation(sig[:], sigin[:], AF.Sigmoid)

    nc.tensor.transpose(Tz, bund[:, 16 : 16 + BL], id_t[:])
    nc.tensor.transpose(Te, bund[:, 32 : 32 + BL], id_t[:])
    Zb = pers.tile([BL, 1], F32)
    nc.vector.tensor_reduce(Zb[:], Tz, AX.X, ALU.add)
    # near-end mass: full Zp rows p in [103,128) plus the p=102 tail sum
    Nbh = pers.tile([BL, 1], F32)
    nc.vector.tensor_reduce(Nbh[:], Tz[:, NE_P + 1 : P], AX.X, ALU.add)
    Nb = pers.tile([BL, 1], F32)
    nc.vector.tensor_add(Nb[:], Nbh[:], Te[:, NE_P : NE_P + 1])
    invZ = pers.tile([BL, 1], F32)
    nc.vector.reciprocal(invZ[:], Zb[:])
    outv = pers.tile([BL, 1], F32)
    nc.vector.scalar_tensor_tensor(
        outv[:], Nb[:], sig[:], invZ[:], ALU.mult, ALU.mult
    )
    nc.sync.dma_start(out[:], outv[:])


# ---------------------------------------------------------------------------
# General path (pm != 0): the proven baseline implementation, f32 throughout.
# ---------------------------------------------------------------------------

BC = 4
NCHUNK = BL // BC
UF_SCHED = [16, 32, 32, 32]
CHUNK_SIZES = [4, 4, 4, 4]
XP_BUFS = 2
WK_BUFS = 3
NE_P0 = 6553 // U
NE_U0 = 6553 - NE_P0 * U


def build_body(ctx, tc, x, params, utc, idc, out, pm_zero=False):
    nc = tc.nc
    pers = ctx.enter_context(tc.tile_pool(name="pers", bufs=1))
    xp = ctx.enter_context(tc.tile_pool(name="xp", bufs=XP_BUFS))
    wk = ctx.enter_context(tc.tile_pool(name="wk", bufs=WK_BUFS))
    psp = ctx.enter_context(tc.tile_pool(name="psp", bufs=2, space="PSUM"))
    ps1 = ctx.enter_context(tc.tile_pool(name="ps1", bufs=1, space="PSUM"))

    ut_t = pers.tile([P, P], F32)
    ones_t = pers.tile([P, P], F32)
    id_t = pers.tile([P, P], F32)
    gt = pers.tile([P, U], F32)
    ptile = pers.tile([P, 3], F32)
    nc.sync.dma_start(ptile[:], params[:])
    nc.gpsimd.memset(ones_t[:], 1.0)
    gti = pers.tile([P, U], mybir.dt.int32)
    nc.gpsimd.iota(gti[:], [[1, U]], base=0, channel_multiplier=U)
    nc.vector.tensor_copy(gt[:], gti[:])

    chunks = []
    o = 0
    for c in CHUNK_SIZES:
        chunks.append((o, c))
        o += c
    assert o == BL
    xts = []
    for ci, (bs, bc) in enumerate(chunks):
        xt = xp.tile([P, bc, U, N], F32, tag="xt")
        src = x[bs : bs + bc].rearrange("b (p u) n -> p b u n", p=P)
        if ci == 0:
            QU = U // 4
            for q in range(4):
                nc.sync.dma_start(
                    xt[:, :, q * QU : (q + 1) * QU, :],
                    src[:, :, q * QU : (q + 1) * QU, :],
                )
        else:
            HU = U // 2
            nc.sync.dma_start(xt[:, :, 0:HU, :], src[:, :, 0:HU, :])
            nc.sync.dma_start(xt[:, :, HU:U, :], src[:, :, HU:U, :])
        if ci == 0:
            nc.sync.dma_start(ut_t[:], utc[:])
            nc.sync.dma_start(id_t[:], idc[:])
        xts.append(xt)

    sv = scalar_prep(nc, pers, ptile)

    def sb(i, np_=P, p0=0):
        return sv[p0 : p0 + np_, i : i + 1]

    nf = pers.tile([P, U], F32)
    nc.vector.tensor_scalar_add(nf[:], gt[:], 1.0)
    zL = pers.tile([P, U], F32)
    nc.vector.tensor_scalar(zL[:], nf[:], sb(1), sb(2), ALU.mult, ALU.add)
    pvnL = pers.tile([P, U], F32)
    nc.vector.reciprocal(pvnL[:], zL[:])
    zR = pers.tile([P, U], F32)
    nc.vector.tensor_scalar(zR[:], gt[:], sb(3), sb(4), ALU.mult, ALU.add)
    pvnR = pers.tile([P, U], F32)
    nc.vector.reciprocal(pvnR[:], zR[:])
    lpvnL = pers.tile([P, U], F32)
    nc.scalar.activation(lpvnL[:], pvnL[:], AF.Ln)
    lpvnR = pers.tile([P, U], F32)
    nc.scalar.activation(lpvnR[:], pvnR[:], AF.Ln)
    kc2 = pers.tile([P, U], F32)
    nc.vector.tensor_add(kc2[:], lpvnL[:], lpvnR[:])

    nRf = pers.tile([P, U], F32)
    nc.vector.tensor_scalar(nRf[:], gt[:], -1.0, 8191.0, ALU.mult, ALU.add)
    gc = pers.tile([P, U], F32)
    nc.vector.tensor_scalar_max(gc[:], gt[:], 1.0)
    inv_n1 = pers.tile([P, U], F32)
    nc.vector.reciprocal(inv_n1[:], gc[:])
    nR1c = pers.tile([P, U], F32)
    nc.vector.tensor_scalar(nR1c[:], gt[:], -1.0, 8190.0, ALU.mult, ALU.add)
    nc.vector.tensor_scalar_max(nR1c[:], nR1c[:], 1.0)
    inv_nR1 = pers.tile([P, U], F32)
    nc.vector.reciprocal(inv_nR1[:], nR1c[:])
    inv_n = pers.tile([P, U], F32)
    nc.vector.reciprocal(inv_n[:], nf[:])
    inv_nR = pers.tile([P, U], F32)
    nRc = pers.tile([P, U], F32)
    nc.vector.tensor_scalar_max(nRc[:], nRf[:], 1.0)
    nc.vector.reciprocal(inv_nR[:], nRc[:])

    n_n1 = pers.tile([P, U], F32)
    nc.vector.tensor_mul(n_n1[:], nf[:], inv_n1[:])
    nR_nR1 = pers.tile([P, U], F32)
    nc.vector.tensor_mul(nR_nR1[:], nRf[:], inv_nR1[:])
    i_nn1 = pers.tile([P, U], F32)
    nc.vector.tensor_mul(i_nn1[:], inv_n[:], inv_n1[:])
    i_nRnR1 = pers.tile([P, U], F32)
    nc.vector.tensor_mul(i_nRnR1[:], inv_nR[:], inv_nR1[:])

    CBL = pers.tile([P, U], F32)
    nc.scalar.activation(CBL[:], n_n1[:], AF.Copy, scale=sb(7))
    CBR = pers.tile([P, U], F32)
    nc.scalar.activation(CBR[:], nR_nR1[:], AF.Copy, scale=sb(7))
    CA2L = pers.tile([P, U], F32)
    q1 = pers.tile([P, U], F32)
    nc.scalar.activation(q1[:], pvnL[:], AF.Copy, scale=sb(8))
    q2 = pers.tile([P, U], F32)
    nc.scalar.activation(q2[:], i_nn1[:], AF.Copy, scale=sb(9))
    nc.vector.tensor_add(CA2L[:], q1[:], q2[:])
    CA2R = pers.tile([P, U], F32)
    q1b = pers.tile([P, U], F32)
    nc.scalar.activation(q1b[:], pvnR[:], AF.Copy, scale=sb(8))
    q2b = pers.tile([P, U], F32)
    nc.scalar.activation(q2b[:], i_nRnR1[:], AF.Copy, scale=sb(9))
    nc.vector.tensor_add(CA2R[:], q1b[:], q2b[:])
    CAL = pers.tile([P, U], F32)
    nc.scalar.activation(CAL[:], pvnL[:], AF.Copy, scale=sb(10))
    CAR = pers.tile([P, U], F32)
    nc.scalar.activation(CAR[:], pvnR[:], AF.Copy, scale=sb(10))
    Cc = pers.tile([P, U], F32)
    p12 = pers.tile([P, U], F32)
    nc.vector.tensor_add(p12[:], pvnL[:], pvnR[:])
    cc1 = pers.tile([P, U], F32)
    nc.scalar.activation(cc1[:], p12[:], AF.Copy, scale=sb(11))
    cct = pers.tile([P, U], F32)
    nc.vector.tensor_scalar(cct[:], kc2[:], 0.5, sb(12), ALU.mult, ALU.add)
    nc.vector.tensor_add(Cc[:], cc1[:], cct[:])
    mlo = pers.tile([P, U], F32)
    nc.vector.tensor_scalar(mlo[:], gt[:], 14.5, NEG, ALU.is_lt, ALU.mult)
    mhi = pers.tile([P, U], F32)
    nc.vector.tensor_scalar(mhi[:], gt[:], 8174.5, NEG, ALU.is_ge, ALU.mult)
    nc.vector.tensor_add(Cc[:], Cc[:], mlo[:])
    nc.vector.tensor_add(Cc[:], Cc[:], mhi[:])
    nemask = pers.tile([P, U], F32)
    nc.vector.tensor_scalar(nemask[:], gt[:], 6552.5, None, ALU.is_ge)

    bund = pers.tile([P, 80], F32)
    zeros = pers.tile([P, max(CHUNK_SIZES) * U], F32)
    nc.gpsimd.memset(zeros[:], 0.0)

    Tall = ps1.tile([BL, 5, P], F32)
    Tm = Tall[:, 0, :]
    Tz = Tall[:, 1, :]
    Te = Tall[:, 2, :]
    Ta = Tall[:, 3, :]
    Tb = Tall[:, 4, :]
    M16 = pers.tile([BL, 1], F32)
    d = pers.tile([BL, P], F32)
    w = pers.tile([BL, P], F32)
    bfW = pers.tile([BL, 1], F32)
    sig = pers.tile([BL, 1], F32)

    def trace_bfw():
        nc.tensor.transpose(Ta, bund[:, 48 : 48 + BL], id_t[:])
        nc.tensor.transpose(Tb, bund[:, 64 : 64 + BL], id_t[:])
        At16 = Ta[:, 0:1]
        Bt16 = Tb[:, 0:1]
        t1 = pers.tile([BL, 1], F32)
        nc.scalar.activation(t1[:], At16, AF.Square, scale=1.0 / 32.0)
        v2 = pers.tile([BL, 1], F32)
        nc.vector.tensor_scalar_mul(v2[:], t1[:], 1.0 / 8192.0)
        vW = pers.tile([BL, 1], F32)
        nc.vector.scalar_tensor_tensor(
            vW[:], Bt16, 1.0 / 1024.0, v2[:], ALU.mult, ALU.subtract
        )
        nc.vector.tensor_scalar(vW[:], vW[:], 1.0 / 8191.0, 1.0e-8, ALU.mult, ALU.max)
        term1 = pers.tile([BL, 1], F32)
        nc.vector.tensor_scalar_mul(term1[:], vW[:], sb(17, BL))
        term2 = pers.tile([BL, 1], F32)
        nc.vector.tensor_scalar_mul(term2[:], t1[:], sb(18, BL))
        uu = pers.tile([BL, 1], F32)
        nc.scalar.activation(uu[:], At16, AF.Identity, bias=sb(6, BL), scale=sb(5, BL))
        u2 = pers.tile([BL, 1], F32)
        nc.scalar.activation(u2[:], uu[:], AF.Square)
        term3 = pers.tile([BL, 1], F32)
        nc.vector.tensor_scalar_mul(term3[:], u2[:], sb(13, BL))
        tsum = pers.tile([BL, 1], F32)
        nc.vector.tensor_add(tsum[:], term1[:], term2[:])
        nc.vector.tensor_sub(tsum[:], tsum[:], term3[:])
        nc.vector.tensor_scalar(bfW[:], tsum[:], -0.5, sb(19, BL), ALU.mult, ALU.add)

    def trace_maxw():
        nc.tensor.transpose(Tm, bund[:, 0:BL], id_t[:])
        nc.vector.tensor_reduce(M16[:], Tm, AX.X, ALU.min, negate=True)
        nc.vector.tensor_scalar(d[:], Tm, -1.0, M16[:], ALU.mult, ALU.subtract)
        nc.scalar.activation(w[:], d[:], AF.Exp)
        sigin = pers.tile([BL, 1], F32)
        nc.vector.tensor_sub(sigin[:], M16[:], bfW[:])
        nc.scalar.activation(sig[:], sigin[:], AF.Sigmoid)

    xhs = [None] * len(chunks)

    def ufof(ci):
        return UF_SCHED[ci] if UF_SCHED is not None else 32

    def trace_fold(ci):
        uf = ufof(ci)
        bc = chunks[ci][1]
        if uf > 0:
            xh = wk.tile([P, bc, uf, 16], F32, tag="xh")
            nc.gpsimd.tensor_add(
                xh[:], xts[ci][:, :, 0:uf, 0:16], xts[ci][:, :, 0:uf, 16:32]
            )
            xhs[ci] = xh

    trace_fold(0)
    for ci, (bs, bc) in enumerate(chunks):
        xt = xts[ci]
        last = ci == len(chunks) - 1
        if not last:
            trace_fold(ci + 1)

        uf = ufof(ci)
        sr = wk.tile([P, bc, U], F32)
        if uf > 0:
            if ci == 0 and uf == 16:
                for q in range(1, 4):
                    nc.vector.tensor_reduce(
                        sr[:, :, q * 16 : (q + 1) * 16],
                        xt[:, :, q * 16 : (q + 1) * 16, :],
                        AX.X,
                        ALU.add,
                    )
            elif uf < U:
                nc.vector.tensor_reduce(
                    sr[:, :, uf:U], xt[:, :, uf:U, :], AX.X, ALU.add
                )
            nc.vector.tensor_reduce(sr[:, :, 0:uf], xhs[ci][:], AX.X, ALU.add)
        else:
            HU = U // 2
            nc.vector.tensor_reduce(
                sr[:, :, 0:HU], xt[:, :, 0:HU, :], AX.X, ALU.add
            )
            nc.vector.tensor_reduce(
                sr[:, :, HU:U], xt[:, :, HU:U, :], AX.X, ALU.add
            )
        sq = wk.tile([P, bc, U], F32)
        nc.scalar.activation(sq[:], sr[:], AF.Square)

        A = wk.tile([P, bc, U], F32)
        nc.vector.tensor_tensor_scan(
            A[:].rearrange("p b u -> p (b u)"),
            sr[:].rearrange("p b u -> p (b u)"),
            zeros[:, 0 : bc * U],
            0.0,
            ALU.add,
            ALU.add,
        )
        Bt_ = wk.tile([P, bc, U], F32)
        nc.vector.tensor_tensor_scan(
            Bt_[:].rearrange("p b u -> p (b u)"),
            sq[:].rearrange("p b u -> p (b u)"),
            zeros[:, 0 : bc * U],
            0.0,
            ALU.add,
            ALU.add,
        )

        rv = wk.tile([P, 2 * bc], F32)
        nc.gpsimd.memset(rv[:, 0:1], 0.0)
        nc.gpsimd.memset(rv[:, bc : bc + 1], 0.0)
        nc.vector.tensor_copy(rv[:, 1:bc], A[:, 0 : bc - 1, U - 1])
        nc.vector.tensor_copy(rv[:, bc + 1 : 2 * bc], Bt_[:, 0 : bc - 1, U - 1])
        ct = wk.tile([P, 2 * bc], F32)
        nc.vector.tensor_sub(ct[:, 0:bc], A[:, :, U - 1], rv[:, 0:bc])
        nc.vector.tensor_sub(ct[:, bc : 2 * bc], Bt_[:, :, U - 1], rv[:, bc : 2 * bc])
        g_ps = psp.tile([P, 2 * bc], F32)
        nc.tensor.matmul(g_ps[:], ut_t[:], ct[:])
        tot_ps = psp.tile([P, 2 * bc], F32)
        nc.tensor.matmul(tot_ps[:], ones_t[:], ct[:])
        off = wk.tile([P, 2 * bc], F32)
        nc.vector.tensor_sub(off[:], g_ps[:], rv[:])

        offA_b = off[:, 0:bc].unsqueeze(2).broadcast_to([P, bc, U])
        offB_b = off[:, bc : 2 * bc].unsqueeze(2).broadcast_to([P, bc, U])
        nc.vector.tensor_add(A[:], A[:], offA_b)
        nc.vector.tensor_add(Bt_[:], Bt_[:], offB_b)

        nc.scalar.copy(bund[:, 48 + bs : 48 + bs + bc], tot_ps[:, 0:bc])
        nc.scalar.copy(bund[:, 64 + bs : 64 + bs + bc], tot_ps[:, bc : 2 * bc])
        if last:
            trace_bfw()
        At_b = (
            bund[:, 48 + bs : 48 + bs + bc].unsqueeze(2).broadcast_to([P, bc, U])
        )
        Btot_b = (
            bund[:, 64 + bs : 64 + bs + bc].unsqueeze(2).broadcast_to([P, bc, U])
        )
        AR = wk.tile([P, bc, U], F32)
        nc.gpsimd.tensor_sub(AR[:], At_b, A[:])
        BR = wk.tile([P, bc, U], F32)
        nc.gpsimd.tensor_sub(BR[:], Btot_b, Bt_[:])

        A2 = wk.tile([P, bc, U], F32)
        nc.scalar.activation(A2[:], A[:], AF.Square)
        AR2 = wk.tile([P, bc, U], F32)
        nc.scalar.activation(AR2[:], AR[:], AF.Square)

        def cb(t):
            return t[:].unsqueeze(1).broadcast_to([P, bc, U])

        bf = wk.tile([P, bc, U], F32)
        p2 = wk.tile([P, bc, U], F32)
        p3 = wk.tile([P, bc, U], F32)
        p5 = wk.tile([P, bc, U], F32)
        p6 = wk.tile([P, bc, U], F32)
        nc.vector.tensor_mul(p2[:], A2[:], cb(CA2L))
        nc.vector.tensor_mul(p3[:], Bt_[:], cb(CBL))
        nc.vector.tensor_mul(p5[:], AR2[:], cb(CA2R))
        (nc.vector if last else nc.gpsimd).tensor_mul(p6[:], BR[:], cb(CBR))
        p1 = wk.tile([P, bc, U], F32)
        p4 = wk.tile([P, bc, U], F32)
        nc.vector.tensor_mul(p1[:], A[:], cb(CAL))
        nc.vector.tensor_mul(p4[:], AR[:], cb(CAR))
        nc.gpsimd.tensor_add(p1[:], p1[:], p2[:])
        nc.vector.tensor_add(p3[:], p3[:], p4[:])
        nc.gpsimd.tensor_add(p5[:], p5[:], p6[:])
        nc.gpsimd.tensor_add(p1[:], p1[:], cb(Cc))
        nc.vector.tensor_add(p3[:], p3[:], p5[:])
        nc.vector.tensor_add(bf[:], p1[:], p3[:])

        nc.vector.tensor_reduce(
            bund[:, bs : bs + bc], bf[:], AX.X, ALU.max, negate=True
        )
        if last:
            trace_maxw()
        e = wk.tile([P, bc, U], F32)
        for b in range(bc):
            nc.scalar.activation(
                e[:, b, :],
                bf[:, b, :],
                AF.Exp,
                bias=bund[:, bs + b : bs + b + 1],
                accum_out=bund[:, 16 + bs + b : 17 + bs + b],
            )
        en = wk.tile([P, bc, U], F32)
        (nc.vector if last else nc.gpsimd).tensor_mul(en[:], e[:], cb(nemask))
        nc.vector.tensor_reduce(
            bund[:, 32 + bs : 32 + bs + bc], en[:], AX.X, ALU.add
        )

    nc.tensor.transpose(Tz, bund[:, 16 : 16 + BL], id_t[:])
    nc.tensor.transpose(Te, bund[:, 32 : 32 + BL], id_t[:])
    wz = pers.tile([BL, P], F32)
    Zb = pers.tile([BL, 1], F32)
    nc.vector.scalar_tensor_tensor(
        wz[:], w[:], 1.0, Tz, ALU.mult, ALU.mult, accum_out=Zb[:]
    )
    wn = pers.tile([BL, P], F32)
    Nb = pers.tile([BL, 1], F32)
    nc.vector.scalar_tensor_tensor(
        wn[:], w[:], 1.0, Te, ALU.mult, ALU.mult, accum_out=Nb[:]
    )
    invZ = pers.tile([BL, 1], F32)
    nc.vector.reciprocal(invZ[:], Zb[:])
    ratio = pers.tile([BL, 1], F32)
    nc.vector.tensor_mul(ratio[:], Nb[:], invZ[:])
    outv = pers.tile([BL, 1], F32)
    nc.vector.tensor_mul(outv[:], sig[:], ratio[:])
    nc.sync.dma_start(out[:], outv[:])


def host_consts():
    ut = np.triu(np.ones((P, P), np.float32), 1)
    ident = np.eye(P, dtype=np.float32)
    return ut, ident


def split_multi_waits(nc):
    """Walrus in this container allows one sync wait per instruction; move
    extra waits onto same-engine NOPs inserted immediately before."""
    import bass_rust

    nid = [0]
    for f in nc.m.functions:
        for b in f.blocks:
            insts = b.instructions
            i = 0
            while i < len(insts):
                ins = insts[i]
                si = ins.sync_info
                if si is not None and si.on_wait is not None and len(si.on_wait) > 1:
                    waits = list(si.on_wait)
                    for w in waits[:-1]:
                        nop = mybir.InstNoOp(
                            name=f"I-waitsplit-{nid[0]}", ins=[], outs=[]
                        )
                        nid[0] += 1
                        nop.engine = ins.engine
                        nop.sync_info = bass_rust.SyncInfo(
                            on_wait=[w], on_update=[]
                        )
                        insts.insert(i, nop)
                        i += 1
                    si.on_wait = waits[-1:]
                i += 1


_NC_CACHE = {}


def build_nc(split=True, reps=1, pm_zero=False):
    global _NC_CACHE
    key = (split, reps, pm_zero)
    if key in _NC_CACHE:
        return _NC_CACHE[key]
    nc = bass.Bass()
    x = nc.declare_dram_parameter("x", [BL, T, N], F32, isOutput=False)
    if pm_zero:
        coef16 = nc.declare_dram_parameter("coef16", [P, 3 * U], F16, isOutput=False)
        coef32 = nc.declare_dram_parameter(
            "coef32", [P, 2 * U + NS], F32, isOutput=False
        )
    else:
        params = nc.declare_dram_parameter("params", [P, 3], F32, isOutput=False)
    utc = nc.declare_dram_parameter("utc", [P, P], F32, isOutput=False)
    idc = nc.declare_dram_parameter("idc", [P, P], F32, isOutput=False)
    out = nc.declare_dram_parameter("out", [BL, 1], F32, isOutput=True)
    with tile.TileContext(nc) as tc:
        for _ in range(reps):
            with ExitStack() as ctx:
                if pm_zero:
                    build_body_pm0(
                        ctx, tc, x[:], coef16[:], coef32[:], utc[:], idc[:], out[:]
                    )
                else:
                    build_body(
                        ctx, tc, x[:], params[:], utc[:], idc[:], out[:]
                    )
    if split:
        split_multi_waits(nc)
    _NC_CACHE[key] = nc
    return nc


def host_coefs(pm, pvar, nvar):
    """Host-side coefficient vectors + scalar slots for the pm==0 fast path."""
    pv = float(np.log1p(np.exp(pvar)))
    nv = float(np.log1p(np.exp(nvar)))
    inv_nv = 1.0 / nv
    inv_pv = 1.0 / pv
    kq2 = 0.5 * inv_nv / 1024.0          # kq/2
    k = inv_nv / 32.0
    g = np.arange(T, dtype=np.float64).reshape(P, U)
    nf = g + 1.0
    nR = 8191.0 - g
    pvnL = 1.0 / (nf * inv_nv + inv_pv)
    pvnR = 1.0 / (nR * inv_nv + inv_pv)
    inv_n1 = 1.0 / np.maximum(g, 1.0)
    inv_nR1 = 1.0 / np.maximum(nR - 1.0, 1.0)
    i_nn1 = inv_n1 / nf
    i_nRnR1 = inv_nR1 / np.maximum(nR, 1.0)
    CA2L = 0.5 * k * k * pvnL + kq2 * i_nn1
    CA2R = 0.5 * k * k * pvnR + kq2 * i_nRnR1
    SBt = -kq2 * (nf * inv_n1 - nR * inv_nR1)
    CBR2 = -kq2 * inv_nR1
    Ccv = 0.5 * (np.log(pvnL) + np.log(pvnR))
    Ccv = np.where(g < LO_THR, Ccv + NEG16, Ccv)
    Ccv = np.where(g >= HI_THR, Ccv + NEG16, Ccv)
    coef16 = np.concatenate(
        [(CA2L * 4096.0), (CA2R * 4096.0), Ccv], axis=1
    ).astype(np.float16)
    sv = np.zeros(NS, dtype=np.float64)
    pvW = 1.0 / (8192.0 * inv_nv + inv_pv)
    l2pinv = math.log(2.0 * math.pi * nv)
    sv[5] = k
    sv[6] = pm * inv_pv
    sv[9] = kq2
    sv[12] = -4096.0 * l2pinv - math.log(pv) - pm * pm * inv_pv
    sv[13] = pvW
    sv[17] = 8192.0 * inv_nv
    sv[18] = inv_nv / 8192.0
    sv[19] = (-4096.0 * l2pinv + 0.5 * (math.log(pvW) - math.log(pv))
              - 0.5 * pm * pm * inv_pv)
    slots = np.tile(sv[None, :], (P, 1))
    coef32 = np.concatenate([SBt, CBR2, slots], axis=1).astype(np.float32)
    return coef16, coef32


def make_in_maps(x, prior_mean, prior_var, noise_var):
    x = np.ascontiguousarray(np.asarray(x, dtype=np.float32))
    pm = float(np.asarray(prior_mean).reshape(-1)[0])
    ut, ident = host_consts()
    if pm == 0.0:
        coef16, coef32 = host_coefs(
            pm, float(prior_var[0]), float(noise_var[0])
        )
        base = {"coef16": coef16, "coef32": coef32, "utc": ut, "idc": ident}
    else:
        params = np.tile(
            np.array(
                [[pm, float(prior_var[0]), float(noise_var[0])]],
                dtype=np.float32,
            ),
            (P, 1),
        )
        base = {"params": params, "utc": ut, "idc": ident}
    in_maps = []
    for c in range(NCORES):
        m = dict(base)
        m["x"] = x[c * BL : (c + 1) * BL]
        in_maps.append(m)
    return in_maps


def kernel(x, prior_mean, prior_var, noise_var):
    from concourse.bass_utils import run_bass_kernel_spmd

    in_maps = make_in_maps(x, prior_mean, prior_var, noise_var)
    nc = build_nc(pm_zero=(float(np.asarray(prior_mean).reshape(-1)[0]) == 0.0))
    res = run_bass_kernel_spmd(nc, in_maps, list(range(NCORES)))
    outs = [np.asarray(res.results[c]["out"]).reshape(BL) for c in range(NCORES)]
    return np.concatenate(outs).astype(np.float32)
